# revision 1
# baseline (speedup 1.0000x reference)
"""Llama GQA attention (B=1, S=2048, E=4096, H=32, KV=8, D=128) on 8 trn2 cores.

Sharding: tensor-parallel over KV groups. Core c owns kv head c and q heads
4c..4c+3: wq/wk/wv output-dim shards, wo input-dim shard. Each core computes a
partial [S, E] output; host sums the 8 partials and adds bo.

Device layout (per core): all activations kept transposed, [feature, seq]:
  q = wq_c.T @ x.T  -> [512, S] (4 head-tiles of [128, S]);  k, v -> [128, S]
  RoPE applied in [D, S] layout via partition-swapped multiply (host passes
  sign-adjusted sin), 1/sqrt(D) folded into q's cos/sin.
  scores.T tile [k 128, q 512] = kr_tile.T-matmul; + maskT tile; Exp (ACT)
  AV: out[D, q] += v_tile.T @ P.T tile; row-sum l via ones-matmul (no max
  subtraction -- scores are O(10) for this distribution, exp is safe in f32).
  O-proj: out[q, E] += o_tile.T @ wo -> natural [S, E] partial, DMA'd out.
Mask is treated as data: host classifies each [128 k, 512 q] tile as
all-masked (skip), all-zero (no add), or mixed (DMA + add).
"""

import sys

sys.path.insert(0, "/opt/trn_rl_repo")

import numpy as np

import concourse.bass as bass  # noqa: F401  (engine types referenced via nc)
import concourse.bacc as bacc
import concourse.mybir as mybir
import concourse.tile as tile
from concourse.bass_utils import run_bass_kernel_spmd
from concourse.masks import make_identity

F32 = mybir.dt.float32
F32R = mybir.dt.float32r
ADD = mybir.AluOpType.add
MULT = mybir.AluOpType.mult
EXP = mybir.ActivationFunctionType.Exp

B, S, E = 1, 2048, 4096
H, KV, D = 32, 8, 128
NCORES = 8
HPC = H // NCORES          # 4 q heads per core
ET = E // 128              # 32 contraction tiles
SC = S // 512              # 4 seq chunks of 512
KT = S // 128              # 16 k tiles of 128
ECH = E // 512             # 8 output E chunks

SKIP, NOMASK, MASKED = 0, 1, 2

_build_cache = {}


def _build(classes, use_bias):
    nc = bacc.Bacc(None, target_bir_lowering=False)

    xT = nc.declare_dram_parameter("xT", [E, S], F32, isOutput=False)
    wq = nc.declare_dram_parameter("wq", [E, HPC * D], F32, isOutput=False)
    wk = nc.declare_dram_parameter("wk", [E, D], F32, isOutput=False)
    wv = nc.declare_dram_parameter("wv", [E, D], F32, isOutput=False)
    wo = nc.declare_dram_parameter("wo", [HPC * D, E], F32, isOutput=False)
    cosq = nc.declare_dram_parameter("cosq", [D, S], F32, isOutput=False)
    sinq = nc.declare_dram_parameter("sinq", [D, S], F32, isOutput=False)
    cosk = nc.declare_dram_parameter("cosk", [D, S], F32, isOutput=False)
    sink = nc.declare_dram_parameter("sink", [D, S], F32, isOutput=False)
    maskT = nc.declare_dram_parameter("maskT", [S, S], F32, isOutput=False)
    if use_bias:
        bq = nc.declare_dram_parameter("bq", [HPC * D], F32, isOutput=False)
        bk = nc.declare_dram_parameter("bk", [D], F32, isOutput=False)
        bv = nc.declare_dram_parameter("bv", [D], F32, isOutput=False)
    out = nc.declare_dram_parameter("out", [S, E], F32, isOutput=True)

    with tile.TileContext(nc) as tc:
        with tc.tile_pool(name="const", bufs=1) as cpool:
            ident = cpool.tile([128, 128], F32)
            make_identity(nc, ident)
            ones_f = cpool.tile([128, 128], F32)
            nc.vector.memset(ones_f, 1.0)
            ones = cpool.tile([128, 128], F32R)
            nc.vector.tensor_copy(out=ones, in_=ones_f)
            if use_bias:
                bq_sb = cpool.tile([128, HPC], F32)
                nc.sync.dma_start(out=bq_sb, in_=bq.rearrange("(h d) -> d h", d=128))
                bk_sb = cpool.tile([128, 1], F32)
                nc.sync.dma_start(out=bk_sb, in_=bk.rearrange("d -> d 1"))
                bv_sb = cpool.tile([128, 1], F32)
                nc.sync.dma_start(out=bv_sb, in_=bv.rearrange("d -> d 1"))

            with tc.tile_pool(name="qkv", bufs=1) as qkvpool:
                # persistent activations for the attention phase
                qr = [qkvpool.tile([128, S], F32R, name=f"qr{h}", tag=f"qr{h}")
                      for h in range(HPC)]
                kr = qkvpool.tile([128, S], F32R, name="kr", tag="kr")
                vT = qkvpool.tile([128, KT, 128], F32R, tag="vT")  # [k%128, kt, D]

                # ---------- phase 1: projections + RoPE + v transpose ----------
                with tc.tile_pool(name="wpool", bufs=1) as wpool:
                    wq_sb = wpool.tile([128, ET, HPC * D], F32R)
                    wk_sb = wpool.tile([128, ET, D], F32R)
                    wv_sb = wpool.tile([128, ET, D], F32R)
                    wq_r = wq.bitcast(F32R).rearrange("(t p) n -> p t n", p=128)
                    wk_r = wk.bitcast(F32R).rearrange("(t p) n -> p t n", p=128)
                    wv_r = wv.bitcast(F32R).rearrange("(t p) n -> p t n", p=128)
                    for eg in range(8):  # split weight loads across DMA queues
                        sl = slice(eg * 4, eg * 4 + 4)
                        nc.sync.dma_start(out=wq_sb[:, sl, :], in_=wq_r[:, sl, :])
                    nc.sync.dma_start(out=wk_sb, in_=wk_r)
                    nc.sync.dma_start(out=wv_sb, in_=wv_r)

                    with (
                        tc.tile_pool(name="xs", bufs=4) as xpool,
                        tc.tile_pool(name="cs", bufs=2) as cspool,
                        tc.tile_pool(name="tp", bufs=3) as tpool,
                        tc.tile_pool(name="p1", bufs=1, space="PSUM") as ppool,
                        tc.tile_pool(name="p1t", bufs=2, space="PSUM") as ptpool,
                    ):
                        for sc in range(SC):
                            ssl = slice(sc * 512, sc * 512 + 512)
                            acc_q = [ppool.tile([128, 512], F32, name=f"aq{h}", tag=f"aq{h}")
                                     for h in range(HPC)]
                            acc_k = ppool.tile([128, 512], F32, name="ak", tag="ak")
                            acc_v = ppool.tile([128, 512], F32, name="av", tag="av")
                            for e in range(ET):
                                xt = xpool.tile([128, 512], F32R, name="xt", tag="xt")
                                nc.sync.dma_start(
                                    out=xt, in_=xT.bitcast(F32R)[e * 128:(e + 1) * 128, ssl])
                                xr = xt
                                st, sp = (e == 0), (e == ET - 1)
                                for h in range(HPC):
                                    nc.tensor.matmul(
                                        acc_q[h],
                                        wq_sb[:, e, h * 128:(h + 1) * 128],
                                        xr, start=st, stop=sp)
                                nc.tensor.matmul(
                                    acc_k, wk_sb[:, e, :], xr,
                                    start=st, stop=sp)
                                nc.tensor.matmul(
                                    acc_v, wv_sb[:, e, :], xr,
                                    start=st, stop=sp)

                            cq = cspool.tile([128, 512], F32, name="cq", tag="cq")
                            sq = cspool.tile([128, 512], F32, name="sq", tag="sq")
                            ck = cspool.tile([128, 512], F32, name="ck", tag="ck")
                            sk = cspool.tile([128, 512], F32, name="sk", tag="sk")
                            nc.sync.dma_start(out=cq, in_=cosq[:, ssl])
                            nc.sync.dma_start(out=sq, in_=sinq[:, ssl])
                            nc.sync.dma_start(out=ck, in_=cosk[:, ssl])
                            nc.sync.dma_start(out=sk, in_=sink[:, ssl])

                            def rope(dst, acc, ct, st_, bias):
                                src = acc
                                if use_bias:
                                    bsrc = tpool.tile([128, 512], F32, name="bsrc", tag="bsrc")
                                    nc.vector.tensor_scalar_add(bsrc, acc, bias)
                                    src = bsrc
                                tmp = tpool.tile([128, 512], F32, name="tmp", tag="tmp")
                                nc.vector.tensor_tensor(
                                    out=tmp[0:64, :], in0=src[64:128, :],
                                    in1=st_[0:64, :], op=MULT)
                                nc.vector.tensor_tensor(
                                    out=tmp[64:128, :], in0=src[0:64, :],
                                    in1=st_[64:128, :], op=MULT)
                                nc.vector.tensor_tensor(
                                    out=dst, in0=src, in1=ct, op=MULT)
                                nc.vector.tensor_tensor(
                                    out=dst, in0=dst, in1=tmp, op=ADD)

                            for h in range(HPC):
                                rope(qr[h][:, ssl], acc_q[h], cq, sq,
                                     bq_sb[:, h:h + 1] if use_bias else None)
                            rope(kr[:, ssl], acc_k, ck, sk,
                                 bk_sb[:, 0:1] if use_bias else None)

                            vtmp = tpool.tile([128, 512], F32, name="vtmp", tag="vtmp")
                            if use_bias:
                                nc.vector.tensor_scalar_add(vtmp, acc_v, bv_sb[:, 0:1])
                            else:
                                nc.scalar.copy(out=vtmp, in_=acc_v)
                            for j in range(4):
                                tpsum = ptpool.tile([128, 128], F32, name="tp", tag="tp")
                                nc.tensor.transpose(
                                    tpsum, vtmp[:, j * 128:(j + 1) * 128], ident)
                                nc.vector.tensor_copy(
                                    out=vT[:, sc * 4 + j, :], in_=tpsum)

                # ---------- phases 2+3: attention + output projection ----------
                with tc.tile_pool(name="wo", bufs=1) as wopool:
                    wo_sb = wopool.tile([128, HPC, E], F32R)
                    wo_r = wo.bitcast(F32R).rearrange("(t p) n -> p t n", p=128)
                    for hg in range(HPC):
                        nc.sync.dma_start(
                            out=wo_sb[:, hg:hg + 1, :], in_=wo_r[:, hg:hg + 1, :])

                    with (
                        tc.tile_pool(name="mt", bufs=2) as mpool,
                        tc.tile_pool(name="ps", bufs=4) as spool,
                        tc.tile_pool(name="ob", bufs=3) as obpool,
                        tc.tile_pool(name="osb", bufs=2) as opool,
                        tc.tile_pool(name="p2s", bufs=4, space="PSUM") as pst,
                        tc.tile_pool(name="p2a", bufs=1, space="PSUM") as pav,
                        tc.tile_pool(name="p3", bufs=2, space="PSUM") as pop,
                    ):
                        for qc in range(SC):
                            qsl = slice(qc * 512, qc * 512 + 512)
                            vis = [kt for kt in range(KT)
                                   if classes[qc][kt] != SKIP]
                            mts = {}
                            for kt in range(KT):
                                if classes[qc][kt] == MASKED:
                                    mi = len(mts)
                                    mt = mpool.tile([128, 512], F32, name=f"m{mi}", tag=f"m{mi}")
                                    nc.sync.dma_start(
                                        out=mt,
                                        in_=maskT[kt * 128:(kt + 1) * 128, qsl])
                                    mts[kt] = mt
                            o_sb = []
                            for h in range(HPC):
                                av = pav.tile([128, 512], F32, name="avp", tag="avp")
                                lp = pav.tile([128, 512], F32, name="lp", tag="lp")
                                for i, kt in enumerate(vis):
                                    stp = pst.tile([128, 512], F32, name="st", tag="st")
                                    nc.tensor.matmul(
                                        stp,
                                        kr[:, kt * 128:(kt + 1) * 128],
                                        qr[h][:, qsl],
                                        start=True, stop=True,
                                        skip_group_check=True)
                                    if classes[qc][kt] == MASKED:
                                        nc.vector.tensor_tensor(
                                            out=stp, in0=stp, in1=mts[kt], op=ADD)
                                    p = spool.tile([128, 512], F32R, name="p", tag="p")
                                    nc.scalar.activation(out=p, in_=stp, func=EXP)
                                    pr = p
                                    fl, ll = (i == 0), (i == len(vis) - 1)
                                    nc.tensor.matmul(
                                        av, vT[:, kt, :], pr,
                                        start=fl, stop=ll, skip_group_check=True)
                                    nc.tensor.matmul(
                                        lp, ones, pr,
                                        start=fl, stop=ll, skip_group_check=True)
                                rl = spool.tile([128, 512], F32, name="rl", tag="rl")
                                nc.vector.reciprocal(rl, lp)
                                ot = opool.tile([128, 512], F32R, name=f"o{h}", tag=f"o{h}")
                                nc.vector.tensor_tensor(
                                    out=ot, in0=av, in1=rl, op=MULT)
                                o_sb.append(ot)

                            for qs4 in range(4):
                                q0 = qc * 512 + qs4 * 128
                                for ec in range(ECH):
                                    op = pop.tile([128, 512], F32, name="op", tag="op")
                                    for h in range(HPC):
                                        nc.tensor.matmul(
                                            op,
                                            o_sb[h][:, qs4 * 128:(qs4 + 1) * 128],
                                            wo_sb[:, h, ec * 512:(ec + 1) * 512],
                                            start=(h == 0), stop=(h == HPC - 1),
                                            skip_group_check=True)
                                    ob = obpool.tile([128, 512], F32, name="ob", tag="ob")
                                    if ec % 2 == 0:
                                        nc.scalar.copy(out=ob, in_=op)
                                    else:
                                        nc.vector.tensor_copy(out=ob, in_=op)
                                    nc.sync.dma_start(
                                        out=out[q0:q0 + 128, ec * 512:(ec + 1) * 512],
                                        in_=ob)

    nc.finalize()
    return nc


def _host_prep(x, mask, position_ids, wq, bq, wk, bk, wv, bv, wo, bo):
    xT = np.ascontiguousarray(x.reshape(S, E).T)
    pos = position_ids.reshape(S).astype(np.float32)
    inv_freq = 1.0 / (10000.0 ** (np.arange(0, D, 2, dtype=np.float32) / D))
    freqs = np.outer(pos, inv_freq)                     # [S, D/2]
    emb = np.concatenate([freqs, freqs], axis=1)        # [S, D]
    cos = np.cos(emb).astype(np.float32)
    sin = np.sin(emb).astype(np.float32)
    sinS = sin.copy()
    sinS[:, : D // 2] *= -1.0                           # sign for partition swap
    scale = 1.0 / np.sqrt(np.float32(D))
    cosqT = np.ascontiguousarray((cos * scale).T)
    sinqT = np.ascontiguousarray((sinS * scale).T)
    coskT = np.ascontiguousarray(cos.T)
    sinkT = np.ascontiguousarray(sinS.T)
    maskT = np.ascontiguousarray(mask.reshape(S, S).T)

    classes = []
    for qc in range(SC):
        row = []
        for kt in range(KT):
            t = maskT[kt * 128:(kt + 1) * 128, qc * 512:qc * 512 + 512]
            if np.all(t <= -1e8):
                row.append(SKIP)
            elif np.all(t == 0.0):
                row.append(NOMASK)
            else:
                row.append(MASKED)
        if all(c == SKIP for c in row):       # fully-masked rows: keep math finite
            row = [MASKED] * KT
        classes.append(tuple(row))
    classes = tuple(classes)

    use_bias = bool(np.any(bq) or np.any(bk) or np.any(bv))
    return xT, cosqT, sinqT, coskT, sinkT, maskT, classes, use_bias


def kernel(x, mask, position_ids, wq, bq, wk, bk, wv, bv, wo, bo):
    (xT, cosqT, sinqT, coskT, sinkT, maskT,
     classes, use_bias) = _host_prep(
        x, mask, position_ids, wq, bq, wk, bk, wv, bv, wo, bo)

    key = (classes, use_bias)
    if key not in _build_cache:
        _build_cache[key] = _build(classes, use_bias)
    nc = _build_cache[key]

    in_maps = []
    for c in range(NCORES):
        qsl = slice(c * HPC * D, (c + 1) * HPC * D)
        ksl = slice(c * D, (c + 1) * D)
        m = {
            "xT": xT,
            "wq": np.ascontiguousarray(wq[:, qsl]),
            "wk": np.ascontiguousarray(wk[:, ksl]),
            "wv": np.ascontiguousarray(wv[:, ksl]),
            "wo": np.ascontiguousarray(wo[qsl, :]),
            "cosq": cosqT, "sinq": sinqT, "cosk": coskT, "sink": sinkT,
            "maskT": maskT,
        }
        if use_bias:
            m["bq"] = np.ascontiguousarray(bq[qsl])
            m["bk"] = np.ascontiguousarray(bk[ksl])
            m["bv"] = np.ascontiguousarray(bv[ksl])
        in_maps.append(m)

    res = run_bass_kernel_spmd(nc, in_maps, list(range(NCORES)))
    kernel._last_results = res

    acc = res.results[0]["out"].astype(np.float32)
    for c in range(1, NCORES):
        acc = acc + res.results[c]["out"]
    acc = acc + bo[None, :]
    return acc.reshape(B, S, E).astype(np.float32)



# revision 38
# speedup vs baseline: 1.4202x; 1.4202x over previous
"""Llama GQA attention (B=1, S=2048, E=4096, H=32, KV=8, D=128) on 8 trn2 cores.

Sharding: tensor-parallel over KV groups. Core c owns kv head c and q heads
4c..4c+3: wq/wk/wv output-dim shards, wo input-dim shard. Each core computes a
partial [S, E] output (bf16); host sums the 8 partials and adds bo.

Data plane is bf16 (PE runs bf16 at 1 cycle/row for any moving width; DMA
traffic halves vs f32). PSUM accumulation stays f32; RoPE uses f32 cos/sin.
1/sqrt(D) is folded into wq on the host, so q and k share one cos/sin pair.

Per core, everything transposed [feature, seq]:
  phase 1 (per 512-seq chunk, two passes over resident x tiles so the acc
  PSUM banks double-buffer): q = wq_c.T @ x.T -> 4x [128, S]; k, v -> [128, S].
  RoPE via partition-swapped multiply (host passes sign-adjusted sin).
  v transposed via PE into vTo [k, kt, 129] with a constant 1.0 in col 128.
  phase 2: scoresT tile [k 128, q<=512] = kr.T-matmul; diagonal tiles compute
  only the visible q range and add one shared [128,128] triangle mask; Exp on
  ACT -> P bf16. AV is flipped: stationary = P[:, qs*128:+128], moving =
  vTo[:, kt, 0:129] -> av2 [q 128, 129] accumulates over kt in PSUM, col 128
  = softmax denominator for free. Per-partition reciprocal + scalar-mul
  normalize, PE-transpose back to [D, q] for the o-projection.
  phase 3: out[q, E] += o_tile.T @ wo, interleaved one chunk behind attention
  so the PE keeps busy while ACT catches up on Exp.
PSUM is managed as 8 explicit bank tags in one pool (no pool-boundary stalls):
b0-b2 proj passA / scores+..., b3-b5 proj passB / AV accumulators,
b6-b7 v-transposes / o-proj.
"""

import sys

sys.path.insert(0, "/opt/trn_rl_repo")

import numpy as np
import ml_dtypes

import concourse.bass as bass  # noqa: F401
import concourse.bacc as bacc
import concourse.mybir as mybir
import concourse.tile as tile
from concourse.bass_utils import run_bass_kernel_spmd
from concourse.masks import make_identity

F32 = mybir.dt.float32
BF16 = mybir.dt.bfloat16
ADD = mybir.AluOpType.add
MULT = mybir.AluOpType.mult
EXP = mybir.ActivationFunctionType.Exp
BF = ml_dtypes.bfloat16

B, S, E = 1, 2048, 4096
H, KV, D = 32, 8, 128
NCORES = 8
HPC = H // NCORES          # 4 q heads per core
ET = E // 128              # 32 contraction tiles
SC = S // 512              # 4 seq chunks of 512
KT = S // 128              # 16 k tiles of 128
ECH = E // 512             # 8 output E chunks
NEG = -1e9

SKIP = "S"
NOMASK = "N"

_build_cache = {}


def _build(classes, n_mtiles, use_bias):
    nc = bacc.Bacc(None, target_bir_lowering=False)

    xT = nc.declare_dram_parameter("xT", [E, S], BF16, isOutput=False)
    # wab: [q0|q1|v | q2|q3|k] so pass A's half loads first
    wab = nc.declare_dram_parameter("wab", [E, 6 * D], BF16, isOutput=False)
    wo = nc.declare_dram_parameter("wo", [HPC * D, E], BF16, isOutput=False)
    cos = nc.declare_dram_parameter("cos", [D, S], F32, isOutput=False)
    sinS = nc.declare_dram_parameter("sinS", [D, S], F32, isOutput=False)
    # causal triangle as a rank-128 product: triA.T @ triB = -1e9*(k-q)*[k>q]
    triA = nc.declare_dram_parameter("triA", [D, D], BF16, isOutput=False)
    triB = nc.declare_dram_parameter("triB", [D, D], BF16, isOutput=False)
    if n_mtiles:
        mtiles = nc.declare_dram_parameter(
            "mtiles", [n_mtiles * 128, 512], F32, isOutput=False)
    if use_bias:
        bq = nc.declare_dram_parameter("bq", [HPC * D], F32, isOutput=False)
        bk = nc.declare_dram_parameter("bk", [D], F32, isOutput=False)
        bv = nc.declare_dram_parameter("bv", [D], F32, isOutput=False)
    out = nc.declare_dram_parameter("out", [S, E], BF16, isOutput=True)

    wab_r = wab.rearrange("(t p) n -> p t n", p=128)
    wo_r = wo.rearrange("(t p) n -> p t n", p=128)
    xT_r = xT.rearrange("(t p) s -> p t s", p=128)

    with tile.TileContext(nc) as tc:
        with (
            tc.tile_pool(name="const", bufs=1) as cpool,
            tc.tile_pool(name="qkv", bufs=1) as qkvpool,
            tc.tile_pool(name="wts", bufs=1) as wpool,
            tc.tile_pool(name="xres", bufs=2) as xpool,
            tc.tile_pool(name="cs", bufs=2) as cspool,
            tc.tile_pool(name="tp", bufs=1) as tpool,
            tc.tile_pool(name="ps", bufs=4) as spool,
            tc.tile_pool(name="osb", bufs=2) as opool,
            tc.tile_pool(name="onrm", bufs=3) as onpool,
            tc.tile_pool(name="ob", bufs=4) as obpool,
            tc.tile_pool(name="psum", bufs=1, space="PSUM") as P,
        ):
            ident = cpool.tile([128, 128], BF16)
            make_identity(nc, ident)
            triA_sb = cpool.tile([128, 128], BF16)
            triB_sb = cpool.tile([128, 128], BF16)  # DMA'd after chunk-0 loads
            mt_sb = None
            if n_mtiles:
                mt_sb = cpool.tile([128, n_mtiles, 512], F32)
                nc.sync.dma_start(
                    out=mt_sb,
                    in_=mtiles.rearrange("(t p) n -> p t n", p=128))
            if use_bias:
                bq_sb = cpool.tile([128, HPC], F32)
                nc.sync.dma_start(out=bq_sb, in_=bq.rearrange("(h d) -> d h", d=128))
                bk_sb = cpool.tile([128, 1], F32)
                nc.sync.dma_start(out=bk_sb, in_=bk.rearrange("d -> d 1"))
                bv_sb = cpool.tile([128, 1], F32)
                nc.sync.dma_start(out=bv_sb, in_=bv.rearrange("d -> d 1"))

            # persistent activations
            qr = [qkvpool.tile([128, S], BF16, name=f"qr{h}", tag=f"qr{h}")
                  for h in range(HPC)]
            kr = qkvpool.tile([128, S], BF16, name="kr", tag="kr")
            vTo = qkvpool.tile([128, KT, 129], BF16, tag="vTo")  # [k%128, kt, D|1]
            nc.vector.memset(vTo[:, :, 128:129], 1.0)

            wab_sb = wpool.tile([128, ET, 6 * D], BF16)
            wo_sb = wpool.tile([128, HPC, E], BF16)

            # ---------------- phase 1: projections + RoPE + vT ----------------
            def rope(dst, acc, ct, st_, bias):
                src = acc
                if use_bias:
                    bsrc = tpool.tile([128, 512], F32, name="bsrc", tag="bsrc")
                    nc.vector.tensor_scalar_add(bsrc, acc, bias)
                    src = bsrc
                tmp = tpool.tile([128, 512], F32, name="tmp", tag="tmp")
                nc.vector.tensor_tensor(
                    out=tmp[0:64, :], in0=src[64:128, :], in1=st_[0:64, :], op=MULT)
                nc.vector.tensor_tensor(
                    out=tmp[64:128, :], in0=src[0:64, :], in1=st_[64:128, :], op=MULT)
                tmp2 = tpool.tile([128, 512], F32, name="tmp2", tag="tmp2")
                nc.vector.tensor_tensor(out=tmp2, in0=src, in1=ct, op=MULT)
                nc.vector.tensor_tensor(out=dst, in0=tmp2, in1=tmp, op=ADD)

            BA = ["b0", "b1", "b2"]
            BB = ["b3", "b4", "b5"]
            for c in range(SC):
                ssl = slice(c * 512, c * 512 + 512)
                xc = xpool.tile([128, ET, 512], BF16, name="xc", tag="xc")
                # DMA in consumption order, 8-e groups (HWDGE issue is ~625ns
                # per dma_start — few big DMAs, not many small ones)
                cq = cspool.tile([128, 512], F32, name="cq", tag="cq")
                sq = cspool.tile([128, 512], F32, name="sq", tag="sq")
                groups = ([(0, 2), (2, 8), (8, 16), (16, 24), (24, 32)]
                          if c == 0 else [(0, 8), (8, 16), (16, 24), (24, 32)])
                for gi, (e0, e1) in enumerate(groups):
                    eg = slice(e0, e1)
                    if c == 0:
                        nc.sync.dma_start(
                            out=wab_sb[:, eg, 0:384], in_=wab_r[:, eg, 0:384])
                    nc.sync.dma_start(out=xc[:, eg, :], in_=xT_r[:, eg, ssl])
                    if c == 0 and gi == 2:
                        nc.sync.dma_start(out=cq, in_=cos[:, ssl])
                        nc.sync.dma_start(out=sq, in_=sinS[:, ssl])
                if c == 0:
                    for g in range(4):
                        eg = slice(g * 8, g * 8 + 8)
                        nc.sync.dma_start(
                            out=wab_sb[:, eg, 384:768], in_=wab_r[:, eg, 384:768])
                    nc.sync.dma_start(out=triA_sb, in_=triA[:, :])
                    nc.sync.dma_start(out=triB_sb, in_=triB[:, :])
                else:
                    nc.sync.dma_start(out=cq, in_=cos[:, ssl])
                    nc.sync.dma_start(out=sq, in_=sinS[:, ssl])
                if c in (1, 2):
                    for hg in ((0, 1) if c == 1 else (2, 3)):
                        nc.sync.dma_start(
                            out=wo_sb[:, hg:hg + 1, :], in_=wo_r[:, hg:hg + 1, :])

                # pass A: q0, q1, v  (v first so its transpose chain overlaps
                # pass B; the phase-1 tail is then only DVE RoPE)
                accs = [P.tile([128, 512], F32, name=f"pa{i}", tag=BA[i])
                        for i in range(3)]
                for e in range(ET):
                    st, sp = (e == 0), (e == ET - 1)
                    nc.tensor.matmul(accs[0], wab_sb[:, e, 0:128], xc[:, e, :],
                                     start=st, stop=sp)
                    nc.tensor.matmul(accs[1], wab_sb[:, e, 128:256], xc[:, e, :],
                                     start=st, stop=sp)
                    nc.tensor.matmul(accs[2], wab_sb[:, e, 256:384], xc[:, e, :],
                                     start=st, stop=sp)
                rope(qr[0][:, ssl], accs[0], cq, sq,
                     bq_sb[:, 0:1] if use_bias else None)
                rope(qr[1][:, ssl], accs[1], cq, sq,
                     bq_sb[:, 1:2] if use_bias else None)
                vtmp = tpool.tile([128, 512], BF16, name="vtmp", tag="vtmp")
                if use_bias:
                    nc.vector.tensor_scalar_add(vtmp, accs[2], bv_sb[:, 0:1])
                else:
                    nc.scalar.copy(out=vtmp, in_=accs[2])
                # pass B: q2, q3, k
                accs2 = [P.tile([128, 512], F32, name=f"pb{i}", tag=BB[i])
                         for i in range(3)]
                for e in range(ET):
                    st, sp = (e == 0), (e == ET - 1)
                    nc.tensor.matmul(accs2[0], wab_sb[:, e, 384:512], xc[:, e, :],
                                     start=st, stop=sp)
                    nc.tensor.matmul(accs2[1], wab_sb[:, e, 512:640], xc[:, e, :],
                                     start=st, stop=sp)
                    nc.tensor.matmul(accs2[2], wab_sb[:, e, 640:768], xc[:, e, :],
                                     start=st, stop=sp)
                    if e == 4:
                        for j in range(4):
                            tb = "b6" if j % 2 == 0 else "b7"
                            tpsum = P.tile([128, 128], BF16, name="tp", tag=tb)
                            nc.tensor.transpose(
                                tpsum, vtmp[:, j * 128:(j + 1) * 128], ident)
                            nc.vector.tensor_copy(
                                out=vTo[:, c * 4 + j, 0:128], in_=tpsum)
                rope(kr[:, ssl], accs2[2], cq, sq,
                     bk_sb[:, 0:1] if use_bias else None)
                rope(qr[2][:, ssl], accs2[0], cq, sq,
                     bq_sb[:, 2:3] if use_bias else None)
                rope(qr[3][:, ssl], accs2[1], cq, sq,
                     bq_sb[:, 3:4] if use_bias else None)

            # ------------- phases 2+3: attention (+interleaved o-proj) -------
            o_bufs = [None, None]  # [qc%2] -> list of 4 o_sb tiles

            def oproj_pair(qcp, qs, ec0, banks=("b6", "b7"), acts=(False, False)):
                """Two consecutive ec units sharing one output DMA."""
                osrc = o_bufs[qcp % 2]
                q0 = qcp * 512 + qs * 128
                ob = obpool.tile([128, 1024], BF16, name="ob", tag="ob")
                for k in range(2):
                    ec = ec0 + k
                    op = P.tile([128, 512], F32, name="op", tag=banks[k])
                    for hh in range(HPC):
                        nc.tensor.matmul(
                            op,
                            osrc[hh][:, qs * 128:(qs + 1) * 128],
                            wo_sb[:, hh, ec * 512:(ec + 1) * 512],
                            start=(hh == 0), stop=(hh == HPC - 1),
                            skip_group_check=True)
                    if acts[k]:
                        nc.scalar.copy(out=ob[:, k * 512:(k + 1) * 512], in_=op)
                    else:
                        nc.vector.tensor_copy(
                            out=ob[:, k * 512:(k + 1) * 512], in_=op)
                nc.sync.dma_start(
                    out=out[q0:q0 + 128, ec0 * 512:(ec0 + 2) * 512], in_=ob)

            for qc in range(SC):
                # qc0 runs while the chunk-3 RoPE tail still reads b3/b4;
                # park its AV accumulators on banks that free earliest
                AVB = (["b2", "b6", "b7", "b5"] if qc == 0
                       else ["b2", "b3", "b4", "b5"])
                cls = classes[qc]
                vis = [kt for kt in range(KT) if cls[kt] != SKIP]
                o_cur = [opool.tile([128, 512], BF16, name=f"o{h}", tag=f"o{h}")
                         for h in range(HPC)]
                o_bufs[qc % 2] = o_cur
                for h in range(HPC):
                    pend_oproj = [0, 2, 4, 6] if qc > 0 else []
                    # per-qs AV accumulation state: kt lists
                    avkts = [[kt for kt in vis
                              if not (isinstance(cls[kt], tuple)
                                      and cls[kt][0] == "T"
                                      and cls[kt][1] > qs)]
                             for qs in range(4)]
                    avseen = [0, 0, 0, 0]
                    av2 = [P.tile([128, 129], F32, name=f"av{qs}", tag=AVB[qs])
                           for qs in range(4)]
                    def emit_av(kt, p):
                        for qs in range(4):
                            if kt not in avkts[qs]:
                                continue
                            first = avseen[qs] == 0
                            avseen[qs] += 1
                            last = avseen[qs] == len(avkts[qs])
                            nc.tensor.matmul(
                                av2[qs],
                                p[:, qs * 128:(qs + 1) * 128],
                                vTo[:, kt, :],
                                start=first, stop=last,
                                skip_group_check=True)

                    stride = max(1, len(vis) // 4)
                    pops = {k * stride + stride - 1 for k in range(4)}
                    prev = None  # (kt, p) — AV runs one tile behind exp
                    for i, kt in enumerate(vis):
                        cl = cls[kt]
                        q0 = cl[1] * 128 if (isinstance(cl, tuple)
                                             and cl[0] == "T") else 0
                        stp = P.tile([128, 512], F32, name="st",
                                     tag="b0" if i % 2 == 0 else "b1")
                        nc.tensor.matmul(
                            stp[:, q0:512],
                            kr[:, kt * 128:(kt + 1) * 128],
                            qr[h][:, qc * 512 + q0: qc * 512 + 512],
                            start=True, stop=True, skip_group_check=True)
                        if isinstance(cl, tuple) and cl[0] == "T":
                            nc.tensor.matmul(
                                stp[:, q0:q0 + 128], triA_sb, triB_sb,
                                start=False, stop=True, skip_group_check=True)
                        elif isinstance(cl, tuple) and cl[0] == "M":
                            nc.vector.tensor_tensor(
                                out=stp, in0=stp, in1=mt_sb[:, cl[1], :], op=ADD)
                        p = spool.tile([128, 512], BF16, name="p", tag="p")
                        nc.scalar.activation(
                            out=p[:, q0:512], in_=stp[:, q0:512], func=EXP)
                        if prev is not None:
                            emit_av(*prev)
                        prev = (kt, p)
                        if pend_oproj and i in pops:
                            oproj_pair(qc - 1, h, pend_oproj.pop(0))
                    emit_av(*prev)
                    for qs in range(4):
                        rl = onpool.tile([128, 1], F32, name="rl", tag="rl")
                        nc.vector.reciprocal(rl, av2[qs][:, 128:129])
                        otn = onpool.tile([128, 128], BF16, name="otn", tag="otn")
                        nc.vector.tensor_scalar_mul(otn, av2[qs][:, 0:128], rl)
                        tp2 = P.tile([128, 128], BF16, name="tp2", tag=AVB[qs])
                        nc.tensor.transpose(tp2, otn, ident)
                        nc.scalar.copy(
                            out=o_cur[h][:, qs * 128:(qs + 1) * 128], in_=tp2)
                        if pend_oproj:
                            oproj_pair(qc - 1, h, pend_oproj.pop(0))
                    for ec0 in pend_oproj:
                        oproj_pair(qc - 1, h, ec0)
            # final o-proj for the last chunk: six banks, copies alternate
            # DVE/ACT (nothing else runs here)
            FB = ["b0", "b1", "b2", "b3", "b4", "b5"]
            for qs in range(4):
                for pi, ec0 in enumerate(range(0, ECH, 2)):
                    u = qs * 4 + pi
                    oproj_pair(SC - 1, qs, ec0,
                               banks=(FB[(2 * u) % 6], FB[(2 * u + 1) % 6]),
                               acts=(False, True))

    nc.finalize()
    return nc


def _host_prep(x, mask, position_ids, wq, bq, wk, bk, wv, bv, wo, bo):
    scale = 1.0 / np.sqrt(np.float32(D))
    xT = np.ascontiguousarray(x.reshape(S, E).T).astype(BF)
    wq_s = (wq * scale).astype(np.float32)
    wk_b = wk.astype(np.float32)
    wv_b = wv.astype(np.float32)
    wo_b = wo.astype(BF)

    pos = position_ids.reshape(S).astype(np.float32)
    inv_freq = 1.0 / (10000.0 ** (np.arange(0, D, 2, dtype=np.float32) / D))
    freqs = np.outer(pos, inv_freq)                     # [S, D/2]
    emb = np.concatenate([freqs, freqs], axis=1)        # [S, D]
    cosT = np.ascontiguousarray(np.cos(emb).astype(np.float32).T)
    sin = np.sin(emb).astype(np.float32)
    sin[:, : D // 2] *= -1.0                            # sign for partition swap
    sinT = np.ascontiguousarray(sin.T)

    maskT = np.ascontiguousarray(mask.reshape(S, S).T)
    # canonical 128x128 triangle: T[k, q] = 0 if q >= k else NEG
    ktri = np.arange(128)[:, None]
    qtri = np.arange(128)[None, :]
    tri = np.where(qtri >= ktri, 0.0, NEG).astype(np.float32)
    # rank-128 factors: (triA.T @ triB)[k, q] = -s^2 (k - q) for k > q, 0 else
    sfac = np.float32(np.sqrt(1e9))
    mtri = np.arange(128)
    triA = np.where(mtri[:, None] < mtri[None, :], -sfac, 0.0).astype(BF)
    triB = np.where(mtri[:, None] >= mtri[None, :], sfac, 0.0).astype(BF)

    classes = []
    muniq = []      # unique general mask tiles
    mkeys = {}

    def mref(t):
        key = t.tobytes()
        if key not in mkeys:
            mkeys[key] = len(muniq)
            muniq.append(t)
        return ("M", mkeys[key])

    for qc in range(SC):
        row = []
        for kt in range(KT):
            t = maskT[kt * 128:(kt + 1) * 128, qc * 512:qc * 512 + 512]
            if np.all(t <= -1e8):
                row.append(SKIP)
                continue
            if np.all(t == 0.0):
                row.append(NOMASK)
                continue
            j = kt - 4 * qc
            if 0 <= j <= 3:
                q0 = j * 128
                ok = (q0 == 0 or np.all(t[:, :q0] <= -1e8))
                ok = ok and np.array_equal(t[:, q0:q0 + 128], tri)
                ok = ok and (q0 + 128 == 512 or np.all(t[:, q0 + 128:] == 0.0))
                if ok:
                    row.append(("T", j))
                    continue
            row.append(mref(t))
        if all(c == SKIP for c in row):       # fully-masked rows: keep finite
            row = [mref(maskT[kt * 128:(kt + 1) * 128,
                              qc * 512:qc * 512 + 512]) for kt in range(KT)]
        classes.append(tuple(row))
    classes = tuple(classes)

    mtiles = np.concatenate(muniq, axis=0) if muniq else None
    use_bias = bool(np.any(bq) or np.any(bk) or np.any(bv))
    return (xT, wq_s, wk_b, wv_b, wo_b, cosT, sinT, triA, triB, mtiles,
            classes, use_bias)


def kernel(x, mask, position_ids, wq, bq, wk, bk, wv, bv, wo, bo):
    (xT, wq_s, wk_b, wv_b, wo_b, cosT, sinT, triA, triB, mtiles,
     classes, use_bias) = _host_prep(
        x, mask, position_ids, wq, bq, wk, bk, wv, bv, wo, bo)

    n_mtiles = 0 if mtiles is None else mtiles.shape[0] // 128
    key = (classes, n_mtiles, use_bias)
    if key not in _build_cache:
        _build_cache[key] = _build(classes, n_mtiles, use_bias)
    nc = _build_cache[key]

    in_maps = []
    for c in range(NCORES):
        qsl = slice(c * HPC * D, (c + 1) * HPC * D)
        ksl = slice(c * D, (c + 1) * D)
        wqc = wq_s[:, qsl]
        # [q0|q1|v | q2|q3|k]: pass-A half first
        wab = np.ascontiguousarray(np.concatenate(
            [wqc[:, 0:256], wv_b[:, ksl], wqc[:, 256:512], wk_b[:, ksl]],
            axis=1)).astype(BF)
        m = {
            "xT": xT,
            "wab": wab,
            "wo": np.ascontiguousarray(wo_b[qsl, :]),
            "cos": cosT, "sinS": sinT, "triA": triA, "triB": triB,
        }
        if mtiles is not None:
            m["mtiles"] = mtiles
        if use_bias:
            m["bq"] = np.ascontiguousarray(bq[qsl]).astype(np.float32)
            m["bk"] = np.ascontiguousarray(bk[ksl]).astype(np.float32)
            m["bv"] = np.ascontiguousarray(bv[ksl]).astype(np.float32)
        in_maps.append(m)

    res = run_bass_kernel_spmd(nc, in_maps, list(range(NCORES)))
    kernel._last_results = res

    acc = res.results[0]["out"].astype(np.float32)
    for c in range(1, NCORES):
        acc = acc + res.results[c]["out"].astype(np.float32)
    acc = acc + bo[None, :]
    return acc.reshape(B, S, E).astype(np.float32)


# revision 58
# speedup vs baseline: 1.4421x; 1.0154x over previous
"""Llama GQA attention (B=1, S=2048, E=4096, H=32, KV=8, D=128) on 8 trn2 cores.

Sharding: tensor-parallel over KV groups. Core c owns kv head c and q heads
4c..4c+3: wq/wk/wv output-dim shards, wo input-dim shard. Each core computes a
partial [S, E] output (bf16); host sums the 8 partials and adds bo.

Data plane is bf16 (PE runs bf16 at 1 cycle/row for any moving width; DMA
traffic halves vs f32). PSUM accumulation stays f32; RoPE uses f32 cos/sin.
1/sqrt(D) is folded into wq on the host, so q and k share one cos/sin pair.

Per core, everything transposed [feature, seq]:
  phase 1 (per 512-seq chunk, two passes over resident x tiles so the acc
  PSUM banks double-buffer): q = wq_c.T @ x.T -> 4x [128, S]; k, v -> [128, S].
  RoPE via partition-swapped multiply (host passes sign-adjusted sin).
  v transposed via PE into vTo [k, kt, 129] with a constant 1.0 in col 128.
  phase 2: scoresT tile [k 128, q<=512] = kr.T-matmul; diagonal tiles compute
  only the visible q range and add one shared [128,128] triangle mask; Exp on
  ACT -> P bf16. AV is flipped: stationary = P[:, qs*128:+128], moving =
  vTo[:, kt, 0:129] -> av2 [q 128, 129] accumulates over kt in PSUM, col 128
  = softmax denominator for free. Per-partition reciprocal + scalar-mul
  normalize, PE-transpose back to [D, q] for the o-projection.
  phase 3: out[q, E] += o_tile.T @ wo, interleaved one chunk behind attention
  so the PE keeps busy while ACT catches up on Exp.
PSUM is managed as 8 explicit bank tags in one pool (no pool-boundary stalls):
b0-b2 proj passA / scores+..., b3-b5 proj passB / AV accumulators,
b6-b7 v-transposes / o-proj.
"""

import sys

sys.path.insert(0, "/opt/trn_rl_repo")

import numpy as np
import ml_dtypes

import concourse.bass as bass  # noqa: F401
import concourse.bacc as bacc
import concourse.mybir as mybir
import concourse.tile as tile
from concourse.bass_utils import run_bass_kernel_spmd
from concourse.masks import make_identity

F32 = mybir.dt.float32
BF16 = mybir.dt.bfloat16
ADD = mybir.AluOpType.add
MULT = mybir.AluOpType.mult
EXP = mybir.ActivationFunctionType.Exp
BF = ml_dtypes.bfloat16

B, S, E = 1, 2048, 4096
H, KV, D = 32, 8, 128
NCORES = 8
HPC = H // NCORES          # 4 q heads per core
ET = E // 128              # 32 contraction tiles
SC = S // 512              # 4 seq chunks of 512
KT = S // 128              # 16 k tiles of 128
ECH = E // 512             # 8 output E chunks
NEG = -1e9

SKIP = "S"
NOMASK = "N"

_build_cache = {}


def _build(classes, n_mtiles, use_bias):
    nc = bacc.Bacc(None, target_bir_lowering=False)

    xT = nc.declare_dram_parameter("xT", [E, S], BF16, isOutput=False)
    # wab: [q0|q1|v | q2|q3|k] so pass A's half loads first
    wab = nc.declare_dram_parameter("wab", [E, 6 * D], BF16, isOutput=False)
    wo = nc.declare_dram_parameter("wo", [HPC * D, E], BF16, isOutput=False)
    cos = nc.declare_dram_parameter("cos", [D, S], F32, isOutput=False)
    sinS = nc.declare_dram_parameter("sinS", [D, S], F32, isOutput=False)
    # causal triangle as a rank-128 product: triA.T @ triB = -1e9*(k-q)*[k>q]
    triA = nc.declare_dram_parameter("triA", [D, D], BF16, isOutput=False)
    triB = nc.declare_dram_parameter("triB", [D, D], BF16, isOutput=False)
    if n_mtiles:
        mtiles = nc.declare_dram_parameter(
            "mtiles", [n_mtiles * 128, 512], F32, isOutput=False)
    if use_bias:
        bq = nc.declare_dram_parameter("bq", [HPC * D], F32, isOutput=False)
        bk = nc.declare_dram_parameter("bk", [D], F32, isOutput=False)
        bv = nc.declare_dram_parameter("bv", [D], F32, isOutput=False)
    out = nc.declare_dram_parameter("out", [S, E], BF16, isOutput=True)

    wab_r = wab.rearrange("(t p) n -> p t n", p=128)
    wo_r = wo.rearrange("(t p) n -> p t n", p=128)
    xT_r = xT.rearrange("(t p) s -> p t s", p=128)

    with tile.TileContext(nc) as tc:
        with (
            tc.tile_pool(name="const", bufs=1) as cpool,
            tc.tile_pool(name="qkv", bufs=1) as qkvpool,
            tc.tile_pool(name="wts", bufs=1) as wpool,
            tc.tile_pool(name="xres", bufs=2) as xpool,
            tc.tile_pool(name="cs", bufs=2) as cspool,
            tc.tile_pool(name="tp", bufs=1) as tpool,
            tc.tile_pool(name="ps", bufs=4) as spool,
            tc.tile_pool(name="osb", bufs=2) as opool,
            tc.tile_pool(name="onrm", bufs=3) as onpool,
            tc.tile_pool(name="ob", bufs=4) as obpool,
            tc.tile_pool(name="psum", bufs=1, space="PSUM") as P,
        ):
            ident = cpool.tile([128, 128], BF16)
            make_identity(nc, ident)
            triA_sb = cpool.tile([128, 128], BF16)
            triB_sb = cpool.tile([128, 128], BF16)  # DMA'd after chunk-0 loads
            mt_sb = None
            if n_mtiles:
                mt_sb = cpool.tile([128, n_mtiles, 512], F32)
                nc.sync.dma_start(
                    out=mt_sb,
                    in_=mtiles.rearrange("(t p) n -> p t n", p=128))
            if use_bias:
                bq_sb = cpool.tile([128, HPC], F32)
                nc.sync.dma_start(out=bq_sb, in_=bq.rearrange("(h d) -> d h", d=128))
                bk_sb = cpool.tile([128, 1], F32)
                nc.sync.dma_start(out=bk_sb, in_=bk.rearrange("d -> d 1"))
                bv_sb = cpool.tile([128, 1], F32)
                nc.sync.dma_start(out=bv_sb, in_=bv.rearrange("d -> d 1"))

            # persistent activations
            qr = [qkvpool.tile([128, S], BF16, name=f"qr{h}", tag=f"qr{h}")
                  for h in range(HPC)]
            kr = qkvpool.tile([128, S], BF16, name="kr", tag="kr")
            vTo = qkvpool.tile([128, KT, 129], BF16, tag="vTo")  # [k%128, kt, D|1]
            nc.vector.memset(vTo[:, :, 128:129], 1.0)

            wab_sb = wpool.tile([128, ET, 6 * D], BF16)
            wo_sb = wpool.tile([128, HPC, E], BF16)

            # ---------------- phase 1: projections + RoPE + vT ----------------
            def rope(dst, acc, ct, st_, bias):
                src = acc
                if use_bias:
                    bsrc = tpool.tile([128, 512], F32, name="bsrc", tag="bsrc")
                    nc.vector.tensor_scalar_add(bsrc, acc, bias)
                    src = bsrc
                tmp = tpool.tile([128, 512], F32, name="tmp", tag="tmp")
                nc.vector.tensor_tensor(
                    out=tmp[0:64, :], in0=src[64:128, :], in1=st_[0:64, :], op=MULT)
                nc.vector.tensor_tensor(
                    out=tmp[64:128, :], in0=src[0:64, :], in1=st_[64:128, :], op=MULT)
                tmp2 = tpool.tile([128, 512], F32, name="tmp2", tag="tmp2")
                nc.vector.tensor_tensor(out=tmp2, in0=src, in1=ct, op=MULT)
                nc.vector.tensor_tensor(out=dst, in0=tmp2, in1=tmp, op=ADD)

            BA = ["b0", "b1", "b2"]
            BB = ["b3", "b4", "b5"]
            def emit_vtrans(c, vtmp):
                for j in range(4):
                    tb = "b6" if j % 2 == 0 else "b7"
                    tpsum = P.tile([128, 128], BF16, name="tp", tag=tb)
                    nc.tensor.transpose(
                        tpsum, vtmp[:, j * 128:(j + 1) * 128], ident)
                    nc.vector.tensor_copy(
                        out=vTo[:, c * 4 + j, 0:128], in_=tpsum)

            pend_vtrans = None  # chunk 0's v-transposes run in chunk 1
            for c in range(SC):
                ssl = slice(c * 512, c * 512 + 512)
                xc = xpool.tile([128, ET, 512], BF16, name="xc", tag="xc")
                # DMA in consumption order (HWDGE issue is ~625ns per
                # dma_start — few big DMAs, not many small ones)
                cq = cspool.tile([128, 512], F32, name="cq", tag="cq")
                sq = cspool.tile([128, 512], F32, name="sq", tag="sq")
                if c == 0:
                    # both weight halves per group: chunk 0 runs passes A+B
                    # interleaved so PE outpaces the DMA ramp
                    edges = [0, 1, 4, 8, 12, 16, 20, 24, 28, 32]
                    for gi in range(len(edges) - 1):
                        eg = slice(edges[gi], edges[gi + 1])
                        nc.sync.dma_start(
                            out=wab_sb[:, eg, :], in_=wab_r[:, eg, :])
                        nc.sync.dma_start(out=xc[:, eg, :], in_=xT_r[:, eg, ssl])
                        if gi == 4:
                            nc.sync.dma_start(out=cq, in_=cos[:, ssl])
                            nc.sync.dma_start(out=sq, in_=sinS[:, ssl])
                    nc.sync.dma_start(out=triA_sb, in_=triA[:, :])
                    nc.sync.dma_start(out=triB_sb, in_=triB[:, :])
                else:
                    for g in range(4):
                        eg = slice(g * 8, g * 8 + 8)
                        nc.sync.dma_start(out=xc[:, eg, :], in_=xT_r[:, eg, ssl])
                    nc.sync.dma_start(out=cq, in_=cos[:, ssl])
                    nc.sync.dma_start(out=sq, in_=sinS[:, ssl])
                    if c in (1, 2):
                        for hg in ((0, 1) if c == 1 else (2, 3)):
                            nc.sync.dma_start(
                                out=wo_sb[:, hg:hg + 1, :],
                                in_=wo_r[:, hg:hg + 1, :])

                accs = [P.tile([128, 512], F32, name=f"pa{i}", tag=BA[i])
                        for i in range(3)]
                accs2 = [P.tile([128, 512], F32, name=f"pb{i}", tag=BB[i])
                         for i in range(3)]
                if c == 0:
                    # single fused pass: 6 matmuls per e
                    for e in range(ET):
                        st, sp = (e == 0), (e == ET - 1)
                        for ai, col in enumerate((0, 128, 256)):
                            nc.tensor.matmul(
                                accs[ai], wab_sb[:, e, col:col + 128],
                                xc[:, e, :], start=st, stop=sp)
                        for ai, col in enumerate((384, 512, 640)):
                            nc.tensor.matmul(
                                accs2[ai], wab_sb[:, e, col:col + 128],
                                xc[:, e, :], start=st, stop=sp)
                else:
                    # pass A: q0, q1, v (v first so its transpose chain
                    # overlaps pass B; the phase-1 tail is then only RoPE)
                    for e in range(ET):
                        st, sp = (e == 0), (e == ET - 1)
                        for ai, col in enumerate((0, 128, 256)):
                            nc.tensor.matmul(
                                accs[ai], wab_sb[:, e, col:col + 128],
                                xc[:, e, :], start=st, stop=sp)
                rope(qr[0][:, ssl], accs[0], cq, sq,
                     bq_sb[:, 0:1] if use_bias else None)
                rope(qr[1][:, ssl], accs[1], cq, sq,
                     bq_sb[:, 1:2] if use_bias else None)
                vtmp = tpool.tile([128, 512], BF16, name="vtmp", tag="vtmp",
                                  bufs=2)
                if use_bias:
                    nc.vector.tensor_scalar_add(vtmp, accs[2], bv_sb[:, 0:1])
                else:
                    nc.scalar.copy(out=vtmp, in_=accs[2])
                if c > 0:
                    # pass B: q2, q3, k
                    for e in range(ET):
                        st, sp = (e == 0), (e == ET - 1)
                        for ai, col in enumerate((384, 512, 640)):
                            nc.tensor.matmul(
                                accs2[ai], wab_sb[:, e, col:col + 128],
                                xc[:, e, :], start=st, stop=sp)
                        if e == 4 and pend_vtrans is not None:
                            emit_vtrans(*pend_vtrans)
                            pend_vtrans = None
                        if e == 8:
                            emit_vtrans(c, vtmp)
                if c == 0:
                    pend_vtrans = (0, vtmp)
                if c == SC - 1:
                    # defer the pass-B RoPE tail: it would block the q-chunk-0
                    # attention's DVE work (in-order engine); emitted after qc0
                    pend_ropes = [
                        (kr[:, ssl], accs2[2],
                         bk_sb[:, 0:1] if use_bias else None),
                        (qr[2][:, ssl], accs2[0],
                         bq_sb[:, 2:3] if use_bias else None),
                        (qr[3][:, ssl], accs2[1],
                         bq_sb[:, 3:4] if use_bias else None),
                    ]
                    pend_rope_cs = (cq, sq)
                else:
                    rope(kr[:, ssl], accs2[2], cq, sq,
                         bk_sb[:, 0:1] if use_bias else None)
                    rope(qr[2][:, ssl], accs2[0], cq, sq,
                         bq_sb[:, 2:3] if use_bias else None)
                    rope(qr[3][:, ssl], accs2[1], cq, sq,
                         bq_sb[:, 3:4] if use_bias else None)

            # ------------- phases 2+3: attention (+interleaved o-proj) -------
            o_bufs = [None, None]  # [qc%2] -> list of 4 o_sb tiles

            def oproj_pair(qcp, qs, ec0, banks=("b6", "b7"), acts=(False, False),
                           split_dma=False):
                """Two consecutive ec units sharing one output DMA."""
                osrc = o_bufs[qcp % 2]
                q0 = qcp * 512 + qs * 128
                ob = obpool.tile([128, 1024], BF16, name="ob", tag="ob")
                for k in range(2):
                    ec = ec0 + k
                    op = P.tile([128, 512], F32, name="op", tag=banks[k])
                    for hh in range(HPC):
                        nc.tensor.matmul(
                            op,
                            osrc[hh][:, qs * 128:(qs + 1) * 128],
                            wo_sb[:, hh, ec * 512:(ec + 1) * 512],
                            start=(hh == 0), stop=(hh == HPC - 1),
                            skip_group_check=True)
                    if acts[k]:
                        nc.scalar.copy(out=ob[:, k * 512:(k + 1) * 512], in_=op)
                    else:
                        nc.vector.tensor_copy(
                            out=ob[:, k * 512:(k + 1) * 512], in_=op)
                    if split_dma:
                        nc.sync.dma_start(
                            out=out[q0:q0 + 128, ec * 512:(ec + 1) * 512],
                            in_=ob[:, k * 512:(k + 1) * 512])
                if not split_dma:
                    nc.sync.dma_start(
                        out=out[q0:q0 + 128, ec0 * 512:(ec0 + 2) * 512], in_=ob)

            for qc in range(SC):
                # qc0 runs before the deferred chunk-3 RoPE tail frees
                # b3/b4/b5: keep it entirely off those banks (qs3 reuses b2
                # after qs0's tail releases it)
                AVB = (["b2", "b6", "b7", "b2"] if qc == 0
                       else ["b2", "b3", "b4", "b5"])
                cls = classes[qc]
                vis = [kt for kt in range(KT) if cls[kt] != SKIP]
                o_cur = [opool.tile([128, 512], BF16, name=f"o{h}", tag=f"o{h}")
                         for h in range(HPC)]
                o_bufs[qc % 2] = o_cur
                for h in range(HPC):
                    pend_oproj = list(range(ECH)) if qc > 0 else []
                    ob_half = [None]  # open ob tile for the current pair

                    def emit_op_unit(ec):
                        """One o-proj ec unit; pairs share an ob tile+DMA."""
                        osrc = o_bufs[(qc - 1) % 2]
                        q0 = (qc - 1) * 512 + h * 128
                        if ec % 2 == 0:
                            ob_half[0] = obpool.tile(
                                [128, 1024], BF16, name="ob", tag="ob")
                        ob = ob_half[0]
                        op = P.tile([128, 512], F32, name="op",
                                    tag="b6" if ec % 2 == 0 else "b7")
                        for hh in range(HPC):
                            nc.tensor.matmul(
                                op,
                                osrc[hh][:, h * 128:(h + 1) * 128],
                                wo_sb[:, hh, ec * 512:(ec + 1) * 512],
                                start=(hh == 0), stop=(hh == HPC - 1),
                                skip_group_check=True)
                        k = ec % 2
                        if qc == 1 or k == 0:
                            nc.scalar.copy(
                                out=ob[:, k * 512:(k + 1) * 512], in_=op)
                        else:
                            nc.vector.tensor_copy(
                                out=ob[:, k * 512:(k + 1) * 512], in_=op)
                        if k == 1:
                            nc.sync.dma_start(
                                out=out[q0:q0 + 128,
                                        (ec - 1) * 512:(ec + 1) * 512],
                                in_=ob)
                    # per-qs AV accumulation state: kt lists
                    avkts = [[kt for kt in vis
                              if not (isinstance(cls[kt], tuple)
                                      and cls[kt][0] == "T"
                                      and cls[kt][1] > qs)]
                             for qs in range(4)]
                    avseen = [0, 0, 0, 0]
                    av2 = [P.tile([128, 129], F32, name=f"av{qs}", tag=AVB[qs])
                           for qs in range(4)]
                    def emit_qs_tail(qs):
                        rl = onpool.tile([128, 1], F32, name="rl", tag="rl")
                        nc.vector.reciprocal(rl, av2[qs][:, 128:129])
                        otn = onpool.tile([128, 128], BF16, name="otn",
                                          tag="otn")
                        nc.vector.tensor_scalar_mul(otn, av2[qs][:, 0:128], rl)
                        tp2 = P.tile([128, 128], BF16, name="tp2", tag=AVB[qs])
                        nc.tensor.transpose(tp2, otn, ident)
                        nc.scalar.copy(
                            out=o_cur[h][:, qs * 128:(qs + 1) * 128], in_=tp2)

                    def emit_av(kt, p):
                        for qs in range(4):
                            if kt not in avkts[qs]:
                                continue
                            first = avseen[qs] == 0
                            avseen[qs] += 1
                            last = avseen[qs] == len(avkts[qs])
                            nc.tensor.matmul(
                                av2[qs],
                                p[:, qs * 128:(qs + 1) * 128],
                                vTo[:, kt, :],
                                start=first, stop=last,
                                skip_group_check=True)
                            if last:
                                if qc == 0:
                                    emit_qs_tail(qs)  # frees b2 for qs3
                                else:
                                    tails.append(qs)

                    pops = {}
                    for k in range(ECH):
                        pops.setdefault((k * len(vis)) // ECH, 0)
                        pops[(k * len(vis)) // ECH] += 1
                    prev = None  # (kt, p) — AV runs one tile behind exp
                    tails = []  # qs normalize/transpose, deferred one unit
                    for i, kt in enumerate(vis):
                        if tails:
                            emit_qs_tail(tails.pop(0))
                        cl = cls[kt]
                        q0 = cl[1] * 128 if (isinstance(cl, tuple)
                                             and cl[0] == "T") else 0
                        stp = P.tile([128, 512], F32, name="st",
                                     tag="b0" if i % 2 == 0 else "b1")
                        nc.tensor.matmul(
                            stp[:, q0:512],
                            kr[:, kt * 128:(kt + 1) * 128],
                            qr[h][:, qc * 512 + q0: qc * 512 + 512],
                            start=True, stop=True, skip_group_check=True)
                        if isinstance(cl, tuple) and cl[0] == "T":
                            nc.tensor.matmul(
                                stp[:, q0:q0 + 128], triA_sb, triB_sb,
                                start=False, stop=True, skip_group_check=True)
                        elif isinstance(cl, tuple) and cl[0] == "M":
                            nc.vector.tensor_tensor(
                                out=stp, in0=stp, in1=mt_sb[:, cl[1], :], op=ADD)
                        p = spool.tile([128, 512], BF16, name="p", tag="p")
                        nc.scalar.activation(
                            out=p[:, q0:512], in_=stp[:, q0:512], func=EXP)
                        if prev is not None:
                            emit_av(*prev)
                        prev = (kt, p)
                        for _ in range(pops.get(i, 0)):
                            if pend_oproj:
                                emit_op_unit(pend_oproj.pop(0))
                    emit_av(*prev)
                    while tails or pend_oproj:
                        if pend_oproj:
                            emit_op_unit(pend_oproj.pop(0))
                        if tails:
                            emit_qs_tail(tails.pop(0))
                if qc == 0:
                    cqd, sqd = pend_rope_cs
                    for dst, acc, bias in pend_ropes:
                        rope(dst, acc, cqd, sqd, bias)
            # final o-proj for the last chunk: six banks, copies alternate
            # DVE/ACT (nothing else runs here)
            FB = ["b0", "b1", "b2", "b3", "b4", "b5"]
            for qs in range(4):
                for pi, ec0 in enumerate(range(0, ECH, 2)):
                    u = qs * 4 + pi
                    oproj_pair(SC - 1, qs, ec0,
                               banks=(FB[(2 * u) % 6], FB[(2 * u + 1) % 6]),
                               acts=(False, True), split_dma=(u == 15))

    nc.finalize()
    return nc


def _host_prep(x, mask, position_ids, wq, bq, wk, bk, wv, bv, wo, bo):
    scale = 1.0 / np.sqrt(np.float32(D))
    xT = np.ascontiguousarray(x.reshape(S, E).T).astype(BF)
    wq_s = (wq * scale).astype(np.float32)
    wk_b = wk.astype(np.float32)
    wv_b = wv.astype(np.float32)
    wo_b = wo.astype(BF)

    pos = position_ids.reshape(S).astype(np.float32)
    inv_freq = 1.0 / (10000.0 ** (np.arange(0, D, 2, dtype=np.float32) / D))
    freqs = np.outer(pos, inv_freq)                     # [S, D/2]
    emb = np.concatenate([freqs, freqs], axis=1)        # [S, D]
    cosT = np.ascontiguousarray(np.cos(emb).astype(np.float32).T)
    sin = np.sin(emb).astype(np.float32)
    sin[:, : D // 2] *= -1.0                            # sign for partition swap
    sinT = np.ascontiguousarray(sin.T)

    maskT = np.ascontiguousarray(mask.reshape(S, S).T)
    # canonical 128x128 triangle: T[k, q] = 0 if q >= k else NEG
    ktri = np.arange(128)[:, None]
    qtri = np.arange(128)[None, :]
    tri = np.where(qtri >= ktri, 0.0, NEG).astype(np.float32)
    # rank-128 factors: (triA.T @ triB)[k, q] = -s^2 (k - q) for k > q, 0 else
    sfac = np.float32(np.sqrt(1e9))
    mtri = np.arange(128)
    triA = np.where(mtri[:, None] < mtri[None, :], -sfac, 0.0).astype(BF)
    triB = np.where(mtri[:, None] >= mtri[None, :], sfac, 0.0).astype(BF)

    classes = []
    muniq = []      # unique general mask tiles
    mkeys = {}

    def mref(t):
        key = t.tobytes()
        if key not in mkeys:
            mkeys[key] = len(muniq)
            muniq.append(t)
        return ("M", mkeys[key])

    for qc in range(SC):
        row = []
        for kt in range(KT):
            t = maskT[kt * 128:(kt + 1) * 128, qc * 512:qc * 512 + 512]
            if np.all(t <= -1e8):
                row.append(SKIP)
                continue
            if np.all(t == 0.0):
                row.append(NOMASK)
                continue
            j = kt - 4 * qc
            if 0 <= j <= 3:
                q0 = j * 128
                ok = (q0 == 0 or np.all(t[:, :q0] <= -1e8))
                ok = ok and np.array_equal(t[:, q0:q0 + 128], tri)
                ok = ok and (q0 + 128 == 512 or np.all(t[:, q0 + 128:] == 0.0))
                if ok:
                    row.append(("T", j))
                    continue
            row.append(mref(t))
        if all(c == SKIP for c in row):       # fully-masked rows: keep finite
            row = [mref(maskT[kt * 128:(kt + 1) * 128,
                              qc * 512:qc * 512 + 512]) for kt in range(KT)]
        classes.append(tuple(row))
    classes = tuple(classes)

    mtiles = np.concatenate(muniq, axis=0) if muniq else None
    use_bias = bool(np.any(bq) or np.any(bk) or np.any(bv))
    return (xT, wq_s, wk_b, wv_b, wo_b, cosT, sinT, triA, triB, mtiles,
            classes, use_bias)


def kernel(x, mask, position_ids, wq, bq, wk, bk, wv, bv, wo, bo):
    (xT, wq_s, wk_b, wv_b, wo_b, cosT, sinT, triA, triB, mtiles,
     classes, use_bias) = _host_prep(
        x, mask, position_ids, wq, bq, wk, bk, wv, bv, wo, bo)

    n_mtiles = 0 if mtiles is None else mtiles.shape[0] // 128
    key = (classes, n_mtiles, use_bias)
    if key not in _build_cache:
        _build_cache[key] = _build(classes, n_mtiles, use_bias)
    nc = _build_cache[key]

    in_maps = []
    for c in range(NCORES):
        qsl = slice(c * HPC * D, (c + 1) * HPC * D)
        ksl = slice(c * D, (c + 1) * D)
        wqc = wq_s[:, qsl]
        # [q0|q1|v | q2|q3|k]: pass-A half first
        wab = np.ascontiguousarray(np.concatenate(
            [wqc[:, 0:256], wv_b[:, ksl], wqc[:, 256:512], wk_b[:, ksl]],
            axis=1)).astype(BF)
        m = {
            "xT": xT,
            "wab": wab,
            "wo": np.ascontiguousarray(wo_b[qsl, :]),
            "cos": cosT, "sinS": sinT, "triA": triA, "triB": triB,
        }
        if mtiles is not None:
            m["mtiles"] = mtiles
        if use_bias:
            m["bq"] = np.ascontiguousarray(bq[qsl]).astype(np.float32)
            m["bk"] = np.ascontiguousarray(bk[ksl]).astype(np.float32)
            m["bv"] = np.ascontiguousarray(bv[ksl]).astype(np.float32)
        in_maps.append(m)

    res = run_bass_kernel_spmd(nc, in_maps, list(range(NCORES)))
    kernel._last_results = res

    acc = res.results[0]["out"].astype(np.float32)
    for c in range(1, NCORES):
        acc = acc + res.results[c]["out"].astype(np.float32)
    acc = acc + bo[None, :]
    return acc.reshape(B, S, E).astype(np.float32)


# revision 72
# speedup vs baseline: 1.4425x; 1.0002x over previous
"""Llama GQA attention (B=1, S=2048, E=4096, H=32, KV=8, D=128) on 8 trn2 cores.

Sharding: tensor-parallel over KV groups. Core c owns kv head c and q heads
4c..4c+3: wq/wk/wv output-dim shards, wo input-dim shard. Each core computes a
partial [S, E] output (bf16); host sums the 8 partials and adds bo.

Data plane is bf16 (PE runs bf16 at 1 cycle/row for any moving width; DMA
traffic halves vs f32). PSUM accumulation stays f32; RoPE uses f32 cos/sin.
1/sqrt(D) is folded into wq on the host, so q and k share one cos/sin pair.

Per core, everything transposed [feature, seq]:
  phase 1 (per 512-seq chunk, two passes over resident x tiles so the acc
  PSUM banks double-buffer): q = wq_c.T @ x.T -> 4x [128, S]; k, v -> [128, S].
  RoPE via partition-swapped multiply (host passes sign-adjusted sin).
  v transposed via PE into vTo [k, kt, 129] with a constant 1.0 in col 128.
  phase 2: scoresT tile [k 128, q<=512] = kr.T-matmul; diagonal tiles compute
  only the visible q range and add one shared [128,128] triangle mask; Exp on
  ACT -> P bf16. AV is flipped: stationary = P[:, qs*128:+128], moving =
  vTo[:, kt, 0:129] -> av2 [q 128, 129] accumulates over kt in PSUM, col 128
  = softmax denominator for free. Per-partition reciprocal + scalar-mul
  normalize, PE-transpose back to [D, q] for the o-projection.
  phase 3: out[q, E] += o_tile.T @ wo, interleaved one chunk behind attention
  so the PE keeps busy while ACT catches up on Exp.
PSUM is managed as 8 explicit bank tags in one pool (no pool-boundary stalls):
b0-b2 proj passA / scores+..., b3-b5 proj passB / AV accumulators,
b6-b7 v-transposes / o-proj.
"""

import sys

sys.path.insert(0, "/opt/trn_rl_repo")

import numpy as np
import ml_dtypes

import concourse.bass as bass  # noqa: F401
import concourse.bacc as bacc
import concourse.mybir as mybir
import concourse.tile as tile
from concourse.bass_utils import run_bass_kernel_spmd
from concourse.masks import make_identity

F32 = mybir.dt.float32
BF16 = mybir.dt.bfloat16
ADD = mybir.AluOpType.add
MULT = mybir.AluOpType.mult
EXP = mybir.ActivationFunctionType.Exp
BF = ml_dtypes.bfloat16

B, S, E = 1, 2048, 4096
H, KV, D = 32, 8, 128
NCORES = 8
HPC = H // NCORES          # 4 q heads per core
ET = E // 128              # 32 contraction tiles
SC = S // 512              # 4 seq chunks of 512
KT = S // 128              # 16 k tiles of 128
ECH = E // 512             # 8 output E chunks
NEG = -1e9

SKIP = "S"
NOMASK = "N"

_build_cache = {}


def _build(classes, n_mtiles, use_bias):
    nc = bacc.Bacc(None, target_bir_lowering=False)

    xT = nc.declare_dram_parameter("xT", [E, S], BF16, isOutput=False)
    # wab: [q0|q1|v | q2|q3|k] so pass A's half loads first
    wab = nc.declare_dram_parameter("wab", [E, 6 * D], BF16, isOutput=False)
    wo = nc.declare_dram_parameter("wo", [HPC * D, E], BF16, isOutput=False)
    cos = nc.declare_dram_parameter("cos", [D, S], F32, isOutput=False)
    sinS = nc.declare_dram_parameter("sinS", [D, S], F32, isOutput=False)
    # causal triangle as a rank-128 product: triA.T @ triB = -1e9*(k-q)*[k>q]
    triA = nc.declare_dram_parameter("triA", [D, D], BF16, isOutput=False)
    triB = nc.declare_dram_parameter("triB", [D, D], BF16, isOutput=False)
    if n_mtiles:
        mtiles = nc.declare_dram_parameter(
            "mtiles", [n_mtiles * 128, 512], F32, isOutput=False)
    if use_bias:
        bq = nc.declare_dram_parameter("bq", [HPC * D], F32, isOutput=False)
        bk = nc.declare_dram_parameter("bk", [D], F32, isOutput=False)
        bv = nc.declare_dram_parameter("bv", [D], F32, isOutput=False)
    out = nc.declare_dram_parameter("out", [S, E], BF16, isOutput=True)

    wab_r = wab.rearrange("(t p) n -> p t n", p=128)
    wo_r = wo.rearrange("(t p) n -> p t n", p=128)
    xT_r = xT.rearrange("(t p) s -> p t s", p=128)

    with tile.TileContext(nc) as tc:
        with (
            tc.tile_pool(name="const", bufs=1) as cpool,
            tc.tile_pool(name="qkv", bufs=1) as qkvpool,
            tc.tile_pool(name="wts", bufs=1) as wpool,
            tc.tile_pool(name="xres", bufs=2) as xpool,
            tc.tile_pool(name="cs", bufs=2) as cspool,
            tc.tile_pool(name="tp", bufs=1) as tpool,
            tc.tile_pool(name="ps", bufs=4) as spool,
            tc.tile_pool(name="osb", bufs=2) as opool,
            tc.tile_pool(name="onrm", bufs=3) as onpool,
            tc.tile_pool(name="ob", bufs=4) as obpool,
            tc.tile_pool(name="psum", bufs=1, space="PSUM") as P,
        ):
            ident = cpool.tile([128, 128], BF16)
            make_identity(nc, ident)
            triA_sb = cpool.tile([128, 128], BF16)
            triB_sb = cpool.tile([128, 128], BF16)  # DMA'd after chunk-0 loads
            mt_sb = None
            if n_mtiles:
                mt_sb = cpool.tile([128, n_mtiles, 512], F32)
                nc.sync.dma_start(
                    out=mt_sb,
                    in_=mtiles.rearrange("(t p) n -> p t n", p=128))
            if use_bias:
                bq_sb = cpool.tile([128, HPC], F32)
                nc.sync.dma_start(out=bq_sb, in_=bq.rearrange("(h d) -> d h", d=128))
                bk_sb = cpool.tile([128, 1], F32)
                nc.sync.dma_start(out=bk_sb, in_=bk.rearrange("d -> d 1"))
                bv_sb = cpool.tile([128, 1], F32)
                nc.sync.dma_start(out=bv_sb, in_=bv.rearrange("d -> d 1"))

            # persistent activations
            qr = [qkvpool.tile([128, S], BF16, name=f"qr{h}", tag=f"qr{h}")
                  for h in range(HPC)]
            kr = qkvpool.tile([128, S], BF16, name="kr", tag="kr")
            vTo = qkvpool.tile([128, KT, 129], BF16, tag="vTo")  # [k%128, kt, D|1]
            nc.vector.memset(vTo[:, :, 128:129], 1.0)

            wab_sb = wpool.tile([128, ET, 6 * D], BF16)
            wo_sb = wpool.tile([128, HPC, E], BF16)

            # ---------------- phase 1: projections + RoPE + vT ----------------
            def rope(dst, acc, ct, st_, bias):
                src = acc
                if use_bias:
                    bsrc = tpool.tile([128, 512], F32, name="bsrc", tag="bsrc")
                    nc.vector.tensor_scalar_add(bsrc, acc, bias)
                    src = bsrc
                tmp = tpool.tile([128, 512], F32, name="tmp", tag="tmp")
                nc.vector.tensor_tensor(
                    out=tmp[0:64, :], in0=src[64:128, :], in1=st_[0:64, :], op=MULT)
                nc.vector.tensor_tensor(
                    out=tmp[64:128, :], in0=src[0:64, :], in1=st_[64:128, :], op=MULT)
                tmp2 = tpool.tile([128, 512], F32, name="tmp2", tag="tmp2")
                nc.vector.tensor_tensor(out=tmp2, in0=src, in1=ct, op=MULT)
                nc.vector.tensor_tensor(out=dst, in0=tmp2, in1=tmp, op=ADD)

            BA = ["b0", "b1", "b2"]
            BB = ["b3", "b4", "b5"]
            def emit_vtrans(c, vtmp):
                for j in range(4):
                    tb = "b6" if j % 2 == 0 else "b7"
                    tpsum = P.tile([128, 128], BF16, name="tp", tag=tb)
                    nc.tensor.transpose(
                        tpsum, vtmp[:, j * 128:(j + 1) * 128], ident)
                    nc.vector.tensor_copy(
                        out=vTo[:, c * 4 + j, 0:128], in_=tpsum)

            pend_vtrans = None  # chunk 0's v-transposes run in chunk 1
            for c in range(SC):
                ssl = slice(c * 512, c * 512 + 512)
                xc = xpool.tile([128, ET, 512], BF16, name="xc", tag="xc")
                # DMA in consumption order (HWDGE issue is ~625ns per
                # dma_start — few big DMAs, not many small ones)
                cq = cspool.tile([128, 512], F32, name="cq", tag="cq")
                sq = cspool.tile([128, 512], F32, name="sq", tag="sq")
                if c == 0:
                    # both weight halves per group: chunk 0 runs passes A+B
                    # interleaved so PE outpaces the DMA ramp
                    edges = [0, 1, 4, 8, 12, 16, 20, 24, 28, 32]
                    for gi in range(len(edges) - 1):
                        eg = slice(edges[gi], edges[gi + 1])
                        nc.sync.dma_start(
                            out=wab_sb[:, eg, :], in_=wab_r[:, eg, :])
                        nc.sync.dma_start(out=xc[:, eg, :], in_=xT_r[:, eg, ssl])
                        if gi == 4:
                            nc.sync.dma_start(out=cq, in_=cos[:, ssl])
                            nc.sync.dma_start(out=sq, in_=sinS[:, ssl])
                    nc.sync.dma_start(out=triA_sb, in_=triA[:, :])
                    nc.sync.dma_start(out=triB_sb, in_=triB[:, :])
                else:
                    for g in range(4):
                        eg = slice(g * 8, g * 8 + 8)
                        nc.sync.dma_start(out=xc[:, eg, :], in_=xT_r[:, eg, ssl])
                    nc.sync.dma_start(out=cq, in_=cos[:, ssl])
                    nc.sync.dma_start(out=sq, in_=sinS[:, ssl])
                    if c in (1, 2):
                        for hg in ((0, 1) if c == 1 else (2, 3)):
                            nc.sync.dma_start(
                                out=wo_sb[:, hg:hg + 1, :],
                                in_=wo_r[:, hg:hg + 1, :])

                accs = [P.tile([128, 512], F32, name=f"pa{i}", tag=BA[i])
                        for i in range(3)]
                accs2 = [P.tile([128, 512], F32, name=f"pb{i}", tag=BB[i])
                         for i in range(3)]
                if c == 0:
                    # single fused pass: 6 matmuls per e
                    for e in range(ET):
                        st, sp = (e == 0), (e == ET - 1)
                        for ai, col in enumerate((0, 128, 256)):
                            nc.tensor.matmul(
                                accs[ai], wab_sb[:, e, col:col + 128],
                                xc[:, e, :], start=st, stop=sp)
                        for ai, col in enumerate((384, 512, 640)):
                            nc.tensor.matmul(
                                accs2[ai], wab_sb[:, e, col:col + 128],
                                xc[:, e, :], start=st, stop=sp)
                else:
                    # pass A: q0, q1, v (v first so its transpose chain
                    # overlaps pass B; the phase-1 tail is then only RoPE)
                    for e in range(ET):
                        st, sp = (e == 0), (e == ET - 1)
                        for ai, col in enumerate((0, 128, 256)):
                            nc.tensor.matmul(
                                accs[ai], wab_sb[:, e, col:col + 128],
                                xc[:, e, :], start=st, stop=sp)
                rope(qr[0][:, ssl], accs[0], cq, sq,
                     bq_sb[:, 0:1] if use_bias else None)
                rope(qr[1][:, ssl], accs[1], cq, sq,
                     bq_sb[:, 1:2] if use_bias else None)
                vtmp = tpool.tile([128, 512], BF16, name="vtmp", tag="vtmp",
                                  bufs=2)
                if use_bias:
                    nc.vector.tensor_scalar_add(vtmp, accs[2], bv_sb[:, 0:1])
                else:
                    nc.scalar.copy(out=vtmp, in_=accs[2])
                if c > 0:
                    # pass B: q2, q3, k
                    for e in range(ET):
                        st, sp = (e == 0), (e == ET - 1)
                        for ai, col in enumerate((384, 512, 640)):
                            nc.tensor.matmul(
                                accs2[ai], wab_sb[:, e, col:col + 128],
                                xc[:, e, :], start=st, stop=sp)
                        if e == 4 and pend_vtrans is not None:
                            emit_vtrans(*pend_vtrans)
                            pend_vtrans = None
                        if e == 8:
                            emit_vtrans(c, vtmp)
                if c == 0:
                    pend_vtrans = (0, vtmp)
                if c == SC - 1:
                    # defer the pass-B RoPE tail: it would block the q-chunk-0
                    # attention's DVE work (in-order engine); emitted after qc0
                    pend_ropes = [
                        (kr[:, ssl], accs2[2],
                         bk_sb[:, 0:1] if use_bias else None),
                        (qr[2][:, ssl], accs2[0],
                         bq_sb[:, 2:3] if use_bias else None),
                        (qr[3][:, ssl], accs2[1],
                         bq_sb[:, 3:4] if use_bias else None),
                    ]
                    pend_rope_cs = (cq, sq)
                else:
                    rope(kr[:, ssl], accs2[2], cq, sq,
                         bk_sb[:, 0:1] if use_bias else None)
                    rope(qr[2][:, ssl], accs2[0], cq, sq,
                         bq_sb[:, 2:3] if use_bias else None)
                    rope(qr[3][:, ssl], accs2[1], cq, sq,
                         bq_sb[:, 3:4] if use_bias else None)

            # ------------- phases 2+3: attention (+interleaved o-proj) -------
            o_bufs = [None, None]  # [qc%2] -> list of 4 o_sb tiles

            def oproj_pair(qcp, qs, ec0, slot=("b6", "b7"),
                           acts=(False, False), split_dma=False):
                """Two consecutive ec units sharing one output DMA."""
                osrc = o_bufs[qcp % 2]
                q0 = qcp * 512 + qs * 128
                ob = obpool.tile([128, 1024], BF16, name="ob", tag="ob")
                for k in range(2):
                    ec = ec0 + k
                    op = P.tile([128, 512], F32, name="op", tag=slot[k])
                    for hh in range(HPC):
                        nc.tensor.matmul(
                            op,
                            osrc[hh][:, qs * 128:(qs + 1) * 128],
                            wo_sb[:, hh, ec * 512:(ec + 1) * 512],
                            start=(hh == 0), stop=(hh == HPC - 1),
                            skip_group_check=True)
                    if acts[k]:
                        nc.scalar.copy(out=ob[:, k * 512:(k + 1) * 512], in_=op)
                    else:
                        nc.vector.tensor_copy(
                            out=ob[:, k * 512:(k + 1) * 512], in_=op)
                    if split_dma:
                        nc.sync.dma_start(
                            out=out[q0:q0 + 128, ec * 512:(ec + 1) * 512],
                            in_=ob[:, k * 512:(k + 1) * 512])
                if not split_dma:
                    nc.sync.dma_start(
                        out=out[q0:q0 + 128, ec0 * 512:(ec0 + 2) * 512], in_=ob)

            for qc in range(SC):
                # qc0 runs before the deferred chunk-3 RoPE tail frees
                # b3/b4/b5: keep it entirely off those banks (qs3 reuses b2
                # after qs0's tail releases it)
                AVB = (["b2", "b6", "b7", "b2"] if qc == 0
                       else ["b2", "b3", "b4", "b5"])
                cls = classes[qc]
                vis = [kt for kt in range(KT) if cls[kt] != SKIP]
                o_cur = [opool.tile([128, 512], BF16, name=f"o{h}", tag=f"o{h}")
                         for h in range(HPC)]
                o_bufs[qc % 2] = o_cur
                for h in range(HPC):
                    pend_oproj = list(range(ECH)) if qc > 0 else []
                    ob_half = [None]  # open ob tile for the current pair

                    def emit_op_unit(ec):
                        """One o-proj ec unit; pairs share an ob tile+DMA."""
                        osrc = o_bufs[(qc - 1) % 2]
                        q0 = (qc - 1) * 512 + h * 128
                        if ec % 2 == 0:
                            ob_half[0] = obpool.tile(
                                [128, 1024], BF16, name="ob", tag="ob")
                        ob = ob_half[0]
                        op = P.tile([128, 512], F32, name="op",
                                    tag="b6" if ec % 2 == 0 else "b7")
                        for hh in range(HPC):
                            nc.tensor.matmul(
                                op,
                                osrc[hh][:, h * 128:(h + 1) * 128],
                                wo_sb[:, hh, ec * 512:(ec + 1) * 512],
                                start=(hh == 0), stop=(hh == HPC - 1),
                                skip_group_check=True)
                        k = ec % 2
                        if qc == 1 or k == 0:
                            nc.scalar.copy(
                                out=ob[:, k * 512:(k + 1) * 512], in_=op)
                        else:
                            nc.vector.tensor_copy(
                                out=ob[:, k * 512:(k + 1) * 512], in_=op)
                        if k == 1:
                            nc.sync.dma_start(
                                out=out[q0:q0 + 128,
                                        (ec - 1) * 512:(ec + 1) * 512],
                                in_=ob)
                    # per-qs AV accumulation state: kt lists
                    avkts = [[kt for kt in vis
                              if not (isinstance(cls[kt], tuple)
                                      and cls[kt][0] == "T"
                                      and cls[kt][1] > qs)]
                             for qs in range(4)]
                    avseen = [0, 0, 0, 0]
                    av2 = [P.tile([128, 129], F32, name=f"av{qs}", tag=AVB[qs])
                           for qs in range(4)]
                    def emit_qs_tail(qs):
                        rl = onpool.tile([128, 1], F32, name="rl", tag="rl")
                        nc.vector.reciprocal(rl, av2[qs][:, 128:129])
                        otn = onpool.tile([128, 128], BF16, name="otn",
                                          tag="otn")
                        nc.vector.tensor_scalar_mul(otn, av2[qs][:, 0:128], rl)
                        tp2 = P.tile([128, 128], BF16, name="tp2", tag=AVB[qs])
                        nc.tensor.transpose(tp2, otn, ident)
                        if qc >= 2:  # ACT is exp-saturated in late chunks
                            nc.vector.tensor_copy(
                                out=o_cur[h][:, qs * 128:(qs + 1) * 128],
                                in_=tp2)
                        else:
                            nc.scalar.copy(
                                out=o_cur[h][:, qs * 128:(qs + 1) * 128],
                                in_=tp2)

                    def emit_av(kt, p):
                        for qs in range(4):
                            if kt not in avkts[qs]:
                                continue
                            first = avseen[qs] == 0
                            avseen[qs] += 1
                            last = avseen[qs] == len(avkts[qs])
                            nc.tensor.matmul(
                                av2[qs],
                                p[:, qs * 128:(qs + 1) * 128],
                                vTo[:, kt, :],
                                start=first, stop=last,
                                skip_group_check=True)
                            if last:
                                if qc == 0:
                                    emit_qs_tail(qs)  # frees b2 for qs3
                                else:
                                    tails.append(qs)

                    pops = {}
                    for k in range(ECH):
                        pops.setdefault((k * len(vis)) // ECH, 0)
                        pops[(k * len(vis)) // ECH] += 1
                    prev = None  # (kt, p) — AV runs one tile behind exp
                    tails = []  # qs normalize/transpose, deferred one unit
                    for i, kt in enumerate(vis):
                        if tails:
                            emit_qs_tail(tails.pop(0))
                        cl = cls[kt]
                        q0 = cl[1] * 128 if (isinstance(cl, tuple)
                                             and cl[0] == "T") else 0
                        stp = P.tile([128, 512], F32, name="st",
                                     tag="b0" if i % 2 == 0 else "b1")
                        nc.tensor.matmul(
                            stp[:, q0:512],
                            kr[:, kt * 128:(kt + 1) * 128],
                            qr[h][:, qc * 512 + q0: qc * 512 + 512],
                            start=True, stop=True, skip_group_check=True)
                        if isinstance(cl, tuple) and cl[0] == "T":
                            nc.tensor.matmul(
                                stp[:, q0:q0 + 128], triA_sb, triB_sb,
                                start=False, stop=True, skip_group_check=True)
                        elif isinstance(cl, tuple) and cl[0] == "M":
                            nc.vector.tensor_tensor(
                                out=stp, in0=stp, in1=mt_sb[:, cl[1], :],
                                op=ADD)
                        p = spool.tile([128, 512], BF16, name="p", tag="p")
                        nc.scalar.activation(
                            out=p[:, q0:512], in_=stp[:, q0:512], func=EXP)
                        if prev is not None:
                            emit_av(*prev)
                        prev = (kt, p)
                        for _ in range(pops.get(i, 0)):
                            if pend_oproj:
                                emit_op_unit(pend_oproj.pop(0))
                    emit_av(*prev)
                    while tails or pend_oproj:
                        if pend_oproj:
                            emit_op_unit(pend_oproj.pop(0))
                        if tails:
                            emit_qs_tail(tails.pop(0))
                if qc == 0:
                    cqd, sqd = pend_rope_cs
                    for dst, acc, bias in pend_ropes:
                        rope(dst, acc, cqd, sqd, bias)
            # final o-proj for the last chunk: six banks, copies alternate
            # DVE/ACT (nothing else runs here)
            FB = ["b0", "b1", "b2", "b3", "b4", "b5"]
            for qs in range(4):
                for pi, ec0 in enumerate(range(0, ECH, 2)):
                    u = qs * 4 + pi
                    oproj_pair(SC - 1, qs, ec0,
                               slot=(FB[(2 * u) % 6], FB[(2 * u + 1) % 6]),
                               acts=(False, True), split_dma=(u >= 14))

    nc.finalize()
    return nc


def _host_prep(x, mask, position_ids, wq, bq, wk, bk, wv, bv, wo, bo):
    scale = 1.0 / np.sqrt(np.float32(D))
    xT = np.ascontiguousarray(x.reshape(S, E).T).astype(BF)
    wq_s = (wq * scale).astype(np.float32)
    wk_b = wk.astype(np.float32)
    wv_b = wv.astype(np.float32)
    wo_b = wo.astype(BF)

    pos = position_ids.reshape(S).astype(np.float32)
    inv_freq = 1.0 / (10000.0 ** (np.arange(0, D, 2, dtype=np.float32) / D))
    freqs = np.outer(pos, inv_freq)                     # [S, D/2]
    emb = np.concatenate([freqs, freqs], axis=1)        # [S, D]
    cosT = np.ascontiguousarray(np.cos(emb).astype(np.float32).T)
    sin = np.sin(emb).astype(np.float32)
    sin[:, : D // 2] *= -1.0                            # sign for partition swap
    sinT = np.ascontiguousarray(sin.T)

    maskT = np.ascontiguousarray(mask.reshape(S, S).T)
    # canonical 128x128 triangle: T[k, q] = 0 if q >= k else NEG
    ktri = np.arange(128)[:, None]
    qtri = np.arange(128)[None, :]
    tri = np.where(qtri >= ktri, 0.0, NEG).astype(np.float32)
    # rank-128 factors: (triA.T @ triB)[k, q] = -s^2 (k - q) for k > q, 0 else
    sfac = np.float32(np.sqrt(1e9))
    mtri = np.arange(128)
    triA = np.where(mtri[:, None] < mtri[None, :], -sfac, 0.0).astype(BF)
    triB = np.where(mtri[:, None] >= mtri[None, :], sfac, 0.0).astype(BF)

    classes = []
    muniq = []      # unique general mask tiles
    mkeys = {}

    def mref(t):
        key = t.tobytes()
        if key not in mkeys:
            mkeys[key] = len(muniq)
            muniq.append(t)
        return ("M", mkeys[key])

    for qc in range(SC):
        row = []
        for kt in range(KT):
            t = maskT[kt * 128:(kt + 1) * 128, qc * 512:qc * 512 + 512]
            if np.all(t <= -1e8):
                row.append(SKIP)
                continue
            if np.all(t == 0.0):
                row.append(NOMASK)
                continue
            j = kt - 4 * qc
            if 0 <= j <= 3:
                q0 = j * 128
                ok = (q0 == 0 or np.all(t[:, :q0] <= -1e8))
                ok = ok and np.array_equal(t[:, q0:q0 + 128], tri)
                ok = ok and (q0 + 128 == 512 or np.all(t[:, q0 + 128:] == 0.0))
                if ok:
                    row.append(("T", j))
                    continue
            row.append(mref(t))
        if all(c == SKIP for c in row):       # fully-masked rows: keep finite
            row = [mref(maskT[kt * 128:(kt + 1) * 128,
                              qc * 512:qc * 512 + 512]) for kt in range(KT)]
        classes.append(tuple(row))
    classes = tuple(classes)

    mtiles = np.concatenate(muniq, axis=0) if muniq else None
    use_bias = bool(np.any(bq) or np.any(bk) or np.any(bv))
    return (xT, wq_s, wk_b, wv_b, wo_b, cosT, sinT, triA, triB, mtiles,
            classes, use_bias)


def kernel(x, mask, position_ids, wq, bq, wk, bk, wv, bv, wo, bo):
    (xT, wq_s, wk_b, wv_b, wo_b, cosT, sinT, triA, triB, mtiles,
     classes, use_bias) = _host_prep(
        x, mask, position_ids, wq, bq, wk, bk, wv, bv, wo, bo)

    n_mtiles = 0 if mtiles is None else mtiles.shape[0] // 128
    key = (classes, n_mtiles, use_bias)
    if key not in _build_cache:
        _build_cache[key] = _build(classes, n_mtiles, use_bias)
    nc = _build_cache[key]

    in_maps = []
    for c in range(NCORES):
        qsl = slice(c * HPC * D, (c + 1) * HPC * D)
        ksl = slice(c * D, (c + 1) * D)
        wqc = wq_s[:, qsl]
        # [q0|q1|v | q2|q3|k]: pass-A half first
        wab = np.ascontiguousarray(np.concatenate(
            [wqc[:, 0:256], wv_b[:, ksl], wqc[:, 256:512], wk_b[:, ksl]],
            axis=1)).astype(BF)
        m = {
            "xT": xT,
            "wab": wab,
            "wo": np.ascontiguousarray(wo_b[qsl, :]),
            "cos": cosT, "sinS": sinT, "triA": triA, "triB": triB,
        }
        if mtiles is not None:
            m["mtiles"] = mtiles
        if use_bias:
            m["bq"] = np.ascontiguousarray(bq[qsl]).astype(np.float32)
            m["bk"] = np.ascontiguousarray(bk[ksl]).astype(np.float32)
            m["bv"] = np.ascontiguousarray(bv[ksl]).astype(np.float32)
        in_maps.append(m)

    res = run_bass_kernel_spmd(nc, in_maps, list(range(NCORES)))
    kernel._last_results = res

    acc = res.results[0]["out"].astype(np.float32)
    for c in range(1, NCORES):
        acc = acc + res.results[c]["out"].astype(np.float32)
    acc = acc + bo[None, :]
    return acc.reshape(B, S, E).astype(np.float32)


# revision 74
# speedup vs baseline: 1.4484x; 1.0042x over previous
"""Llama GQA attention (B=1, S=2048, E=4096, H=32, KV=8, D=128) on 8 trn2 cores.

Sharding: tensor-parallel over KV groups. Core c owns kv head c and q heads
4c..4c+3: wq/wk/wv output-dim shards, wo input-dim shard. Each core computes a
partial [S, E] output (bf16); host sums the 8 partials and adds bo.

Data plane is bf16 (PE runs bf16 at 1 cycle/row for any moving width; DMA
traffic halves vs f32). PSUM accumulation stays f32; RoPE uses f32 cos/sin.
1/sqrt(D) is folded into wq on the host, so q and k share one cos/sin pair.

Per core, everything transposed [feature, seq]:
  phase 1 (per 512-seq chunk, two passes over resident x tiles so the acc
  PSUM banks double-buffer): q = wq_c.T @ x.T -> 4x [128, S]; k, v -> [128, S].
  RoPE via partition-swapped multiply (host passes sign-adjusted sin).
  v transposed via PE into vTo [k, kt, 129] with a constant 1.0 in col 128.
  phase 2: scoresT tile [k 128, q<=512] = kr.T-matmul; diagonal tiles compute
  only the visible q range and add one shared [128,128] triangle mask; Exp on
  ACT -> P bf16. AV is flipped: stationary = P[:, qs*128:+128], moving =
  vTo[:, kt, 0:129] -> av2 [q 128, 129] accumulates over kt in PSUM, col 128
  = softmax denominator for free. Per-partition reciprocal + scalar-mul
  normalize, PE-transpose back to [D, q] for the o-projection.
  phase 3: out[q, E] += o_tile.T @ wo, interleaved one chunk behind attention
  so the PE keeps busy while ACT catches up on Exp.
PSUM is managed as 8 explicit bank tags in one pool (no pool-boundary stalls):
b0-b2 proj passA / scores+..., b3-b5 proj passB / AV accumulators,
b6-b7 v-transposes / o-proj.
"""

import sys

sys.path.insert(0, "/opt/trn_rl_repo")

import numpy as np
import ml_dtypes

import concourse.bass as bass  # noqa: F401
import concourse.bacc as bacc
import concourse.mybir as mybir
import concourse.tile as tile
from concourse.bass_utils import run_bass_kernel_spmd
from concourse.masks import make_identity

F32 = mybir.dt.float32
BF16 = mybir.dt.bfloat16
ADD = mybir.AluOpType.add
MULT = mybir.AluOpType.mult
EXP = mybir.ActivationFunctionType.Exp
BF = ml_dtypes.bfloat16

B, S, E = 1, 2048, 4096
H, KV, D = 32, 8, 128
NCORES = 8
HPC = H // NCORES          # 4 q heads per core
ET = E // 128              # 32 contraction tiles
SC = S // 512              # 4 seq chunks of 512
KT = S // 128              # 16 k tiles of 128
ECH = E // 512             # 8 output E chunks
NEG = -1e9

SKIP = "S"
NOMASK = "N"

_build_cache = {}


def _build(classes, n_mtiles, use_bias):
    nc = bacc.Bacc(None, target_bir_lowering=False)

    xT = nc.declare_dram_parameter("xT", [E, S], BF16, isOutput=False)
    # wab: [q0|q1|v | q2|q3|k] so pass A's half loads first
    wab = nc.declare_dram_parameter("wab", [E, 6 * D], BF16, isOutput=False)
    wo = nc.declare_dram_parameter("wo", [HPC * D, E], BF16, isOutput=False)
    cos = nc.declare_dram_parameter("cos", [D, S], F32, isOutput=False)
    sinS = nc.declare_dram_parameter("sinS", [D, S], F32, isOutput=False)
    # causal triangle as a rank-128 product: triA.T @ triB = -1e9*(k-q)*[k>q]
    triA = nc.declare_dram_parameter("triA", [D, D], BF16, isOutput=False)
    triB = nc.declare_dram_parameter("triB", [D, D], BF16, isOutput=False)
    if n_mtiles:
        mtiles = nc.declare_dram_parameter(
            "mtiles", [n_mtiles * 128, 512], F32, isOutput=False)
    if use_bias:
        bq = nc.declare_dram_parameter("bq", [HPC * D], F32, isOutput=False)
        bk = nc.declare_dram_parameter("bk", [D], F32, isOutput=False)
        bv = nc.declare_dram_parameter("bv", [D], F32, isOutput=False)
    out = nc.declare_dram_parameter("out", [S, E], BF16, isOutput=True)

    wab_r = wab.rearrange("(t p) n -> p t n", p=128)
    wo_r = wo.rearrange("(t p) n -> p t n", p=128)
    xT_r = xT.rearrange("(t p) s -> p t s", p=128)

    with tile.TileContext(nc) as tc:
        with (
            tc.tile_pool(name="const", bufs=1) as cpool,
            tc.tile_pool(name="qkv", bufs=1) as qkvpool,
            tc.tile_pool(name="wts", bufs=1) as wpool,
            tc.tile_pool(name="xres", bufs=2) as xpool,
            tc.tile_pool(name="cs", bufs=2) as cspool,
            tc.tile_pool(name="tp", bufs=1) as tpool,
            tc.tile_pool(name="ps", bufs=4) as spool,
            tc.tile_pool(name="osb", bufs=2) as opool,
            tc.tile_pool(name="onrm", bufs=3) as onpool,
            tc.tile_pool(name="ob", bufs=4) as obpool,
            tc.tile_pool(name="psum", bufs=1, space="PSUM") as P,
        ):
            ident = cpool.tile([128, 128], BF16)
            make_identity(nc, ident)
            triA_sb = cpool.tile([128, 128], BF16)
            triB_sb = cpool.tile([128, 128], BF16)  # DMA'd after chunk-0 loads
            mt_sb = None
            if n_mtiles:
                mt_sb = cpool.tile([128, n_mtiles, 512], F32)
                nc.sync.dma_start(
                    out=mt_sb,
                    in_=mtiles.rearrange("(t p) n -> p t n", p=128))
            if use_bias:
                bq_sb = cpool.tile([128, HPC], F32)
                nc.sync.dma_start(out=bq_sb, in_=bq.rearrange("(h d) -> d h", d=128))
                bk_sb = cpool.tile([128, 1], F32)
                nc.sync.dma_start(out=bk_sb, in_=bk.rearrange("d -> d 1"))
                bv_sb = cpool.tile([128, 1], F32)
                nc.sync.dma_start(out=bv_sb, in_=bv.rearrange("d -> d 1"))

            # persistent activations
            qr = [qkvpool.tile([128, S], BF16, name=f"qr{h}", tag=f"qr{h}")
                  for h in range(HPC)]
            kr = qkvpool.tile([128, S], BF16, name="kr", tag="kr")
            vTo = qkvpool.tile([128, KT, 129], BF16, tag="vTo")  # [k%128, kt, D|1]
            nc.vector.memset(vTo[:, :, 128:129], 1.0)

            wab_sb = wpool.tile([128, ET, 6 * D], BF16)
            wo_sb = wpool.tile([128, HPC, E], BF16)

            # ---------------- phase 1: projections + RoPE + vT ----------------
            def rope(dst, acc, ct, st_, bias):
                src = acc
                if use_bias:
                    bsrc = tpool.tile([128, 512], F32, name="bsrc", tag="bsrc")
                    nc.vector.tensor_scalar_add(bsrc, acc, bias)
                    src = bsrc
                tmp = tpool.tile([128, 512], F32, name="tmp", tag="tmp")
                nc.vector.tensor_tensor(
                    out=tmp[0:64, :], in0=src[64:128, :], in1=st_[0:64, :], op=MULT)
                nc.vector.tensor_tensor(
                    out=tmp[64:128, :], in0=src[0:64, :], in1=st_[64:128, :], op=MULT)
                tmp2 = tpool.tile([128, 512], F32, name="tmp2", tag="tmp2")
                nc.vector.tensor_tensor(out=tmp2, in0=src, in1=ct, op=MULT)
                nc.vector.tensor_tensor(out=dst, in0=tmp2, in1=tmp, op=ADD)

            BA = ["b0", "b1", "b2"]
            BB = ["b3", "b4", "b5"]
            def emit_vtrans(c, vtmp):
                for j in range(4):
                    tb = "b6" if j % 2 == 0 else "b7"
                    tpsum = P.tile([128, 128], BF16, name="tp", tag=tb)
                    nc.tensor.transpose(
                        tpsum, vtmp[:, j * 128:(j + 1) * 128], ident)
                    nc.vector.tensor_copy(
                        out=vTo[:, c * 4 + j, 0:128], in_=tpsum)

            pend_vtrans = None  # chunk 0's v-transposes run in chunk 1
            for c in range(SC):
                ssl = slice(c * 512, c * 512 + 512)
                xc = xpool.tile([128, ET, 512], BF16, name="xc", tag="xc")
                # DMA in consumption order (HWDGE issue is ~625ns per
                # dma_start — few big DMAs, not many small ones)
                cq = cspool.tile([128, 512], F32, name="cq", tag="cq")
                sq = cspool.tile([128, 512], F32, name="sq", tag="sq")
                if c == 0:
                    # both weight halves per group: chunk 0 runs passes A+B
                    # interleaved so PE outpaces the DMA ramp
                    edges = [0, 1, 4, 8, 12, 16, 20, 24, 28, 32]
                    for gi in range(len(edges) - 1):
                        eg = slice(edges[gi], edges[gi + 1])
                        nc.sync.dma_start(
                            out=wab_sb[:, eg, :], in_=wab_r[:, eg, :])
                        nc.sync.dma_start(out=xc[:, eg, :], in_=xT_r[:, eg, ssl])
                        if gi == 4:
                            nc.sync.dma_start(out=cq, in_=cos[:, ssl])
                            nc.sync.dma_start(out=sq, in_=sinS[:, ssl])
                    nc.sync.dma_start(out=triA_sb, in_=triA[:, :])
                    nc.sync.dma_start(out=triB_sb, in_=triB[:, :])
                else:
                    for g in range(4):
                        eg = slice(g * 8, g * 8 + 8)
                        nc.sync.dma_start(out=xc[:, eg, :], in_=xT_r[:, eg, ssl])
                    nc.sync.dma_start(out=cq, in_=cos[:, ssl])
                    nc.sync.dma_start(out=sq, in_=sinS[:, ssl])
                    if c in (1, 2):
                        for hg in ((0, 1) if c == 1 else (2, 3)):
                            nc.sync.dma_start(
                                out=wo_sb[:, hg:hg + 1, :],
                                in_=wo_r[:, hg:hg + 1, :])

                accs = [P.tile([128, 512], F32, name=f"pa{i}", tag=BA[i])
                        for i in range(3)]
                accs2 = [P.tile([128, 512], F32, name=f"pb{i}", tag=BB[i])
                         for i in range(3)]
                def make_vtmp(acc):
                    vt = tpool.tile([128, 512], BF16, name="vtmp", tag="vtmp",
                                    bufs=2)
                    if use_bias:
                        nc.vector.tensor_scalar_add(vt, acc, bv_sb[:, 0:1])
                    else:
                        nc.scalar.copy(out=vt, in_=acc)
                    return vt

                if c == 0:
                    # single fused pass, e-major: chunk 0 is DMA-paced so the
                    # matmuls must chase the per-e loads
                    for e in range(ET):
                        st, sp = (e == 0), (e == ET - 1)
                        for ai, col in enumerate((0, 128, 256)):
                            nc.tensor.matmul(
                                accs[ai], wab_sb[:, e, col:col + 128],
                                xc[:, e, :], start=st, stop=sp)
                        for ai, col in enumerate((384, 512, 640)):
                            nc.tensor.matmul(
                                accs2[ai], wab_sb[:, e, col:col + 128],
                                xc[:, e, :], start=st, stop=sp)
                    rope(qr[0][:, ssl], accs[0], cq, sq,
                         bq_sb[:, 0:1] if use_bias else None)
                    rope(qr[1][:, ssl], accs[1], cq, sq,
                         bq_sb[:, 1:2] if use_bias else None)
                    vtmp = make_vtmp(accs[2])
                    pend_vtrans = (0, vtmp)
                    rope(kr[:, ssl], accs2[2], cq, sq,
                         bk_sb[:, 0:1] if use_bias else None)
                    rope(qr[2][:, ssl], accs2[0], cq, sq,
                         bq_sb[:, 2:3] if use_bias else None)
                    rope(qr[3][:, ssl], accs2[1], cq, sq,
                         bq_sb[:, 3:4] if use_bias else None)
                else:
                    # acc-major: each accumulator finishes early so its RoPE /
                    # copy overlaps the next accumulation instead of tailing
                    def acc_loop(acc, col):
                        for e in range(ET):
                            nc.tensor.matmul(
                                acc, wab_sb[:, e, col:col + 128], xc[:, e, :],
                                start=(e == 0), stop=(e == ET - 1))
                    acc_loop(accs[2], 256)                     # v
                    vtmp = make_vtmp(accs[2])
                    if pend_vtrans is not None:
                        emit_vtrans(*pend_vtrans)
                        pend_vtrans = None
                    acc_loop(accs[0], 0)                       # q0
                    rope(qr[0][:, ssl], accs[0], cq, sq,
                         bq_sb[:, 0:1] if use_bias else None)
                    acc_loop(accs[1], 128)                     # q1
                    rope(qr[1][:, ssl], accs[1], cq, sq,
                         bq_sb[:, 1:2] if use_bias else None)
                    emit_vtrans(c, vtmp)
                    if c == SC - 1:
                        acc_loop(accs2[2], 640)                # k first
                        rope(kr[:, ssl], accs2[2], cq, sq,
                             bk_sb[:, 0:1] if use_bias else None)
                        acc_loop(accs2[0], 384)                # q2
                        acc_loop(accs2[1], 512)                # q3
                        # defer the last two RoPEs: they would block q-chunk-0
                        # attention's DVE work (in-order engine)
                        pend_ropes = [
                            (qr[2][:, ssl], accs2[0],
                             bq_sb[:, 2:3] if use_bias else None),
                            (qr[3][:, ssl], accs2[1],
                             bq_sb[:, 3:4] if use_bias else None),
                        ]
                        pend_rope_cs = (cq, sq)
                    else:
                        acc_loop(accs2[0], 384)                # q2
                        rope(qr[2][:, ssl], accs2[0], cq, sq,
                             bq_sb[:, 2:3] if use_bias else None)
                        acc_loop(accs2[1], 512)                # q3
                        rope(qr[3][:, ssl], accs2[1], cq, sq,
                             bq_sb[:, 3:4] if use_bias else None)
                        acc_loop(accs2[2], 640)                # k
                        rope(kr[:, ssl], accs2[2], cq, sq,
                             bk_sb[:, 0:1] if use_bias else None)

            # ------------- phases 2+3: attention (+interleaved o-proj) -------
            o_bufs = [None, None]  # [qc%2] -> list of 4 o_sb tiles

            def oproj_pair(qcp, qs, ec0, slot=("b6", "b7"),
                           acts=(False, False), split_dma=False):
                """Two consecutive ec units sharing one output DMA."""
                osrc = o_bufs[qcp % 2]
                q0 = qcp * 512 + qs * 128
                ob = obpool.tile([128, 1024], BF16, name="ob", tag="ob")
                for k in range(2):
                    ec = ec0 + k
                    op = P.tile([128, 512], F32, name="op", tag=slot[k])
                    for hh in range(HPC):
                        nc.tensor.matmul(
                            op,
                            osrc[hh][:, qs * 128:(qs + 1) * 128],
                            wo_sb[:, hh, ec * 512:(ec + 1) * 512],
                            start=(hh == 0), stop=(hh == HPC - 1),
                            skip_group_check=True)
                    if acts[k]:
                        nc.scalar.copy(out=ob[:, k * 512:(k + 1) * 512], in_=op)
                    else:
                        nc.vector.tensor_copy(
                            out=ob[:, k * 512:(k + 1) * 512], in_=op)
                    if split_dma:
                        nc.sync.dma_start(
                            out=out[q0:q0 + 128, ec * 512:(ec + 1) * 512],
                            in_=ob[:, k * 512:(k + 1) * 512])
                if not split_dma:
                    nc.sync.dma_start(
                        out=out[q0:q0 + 128, ec0 * 512:(ec0 + 2) * 512], in_=ob)

            for qc in range(SC):
                # qc0 runs before the deferred chunk-3 RoPE tail frees
                # b3/b4/b5: keep it entirely off those banks (qs3 reuses b2
                # after qs0's tail releases it)
                AVB = (["b2", "b6", "b7", "b2"] if qc == 0
                       else ["b2", "b3", "b4", "b5"])
                cls = classes[qc]
                vis = [kt for kt in range(KT) if cls[kt] != SKIP]
                o_cur = [opool.tile([128, 512], BF16, name=f"o{h}", tag=f"o{h}")
                         for h in range(HPC)]
                o_bufs[qc % 2] = o_cur
                for h in range(HPC):
                    pend_oproj = list(range(ECH)) if qc > 0 else []
                    ob_half = [None]  # open ob tile for the current pair

                    def emit_op_unit(ec):
                        """One o-proj ec unit; pairs share an ob tile+DMA."""
                        osrc = o_bufs[(qc - 1) % 2]
                        q0 = (qc - 1) * 512 + h * 128
                        if ec % 2 == 0:
                            ob_half[0] = obpool.tile(
                                [128, 1024], BF16, name="ob", tag="ob")
                        ob = ob_half[0]
                        op = P.tile([128, 512], F32, name="op",
                                    tag="b6" if ec % 2 == 0 else "b7")
                        for hh in range(HPC):
                            nc.tensor.matmul(
                                op,
                                osrc[hh][:, h * 128:(h + 1) * 128],
                                wo_sb[:, hh, ec * 512:(ec + 1) * 512],
                                start=(hh == 0), stop=(hh == HPC - 1),
                                skip_group_check=True)
                        k = ec % 2
                        if qc == 1 or k == 0:
                            nc.scalar.copy(
                                out=ob[:, k * 512:(k + 1) * 512], in_=op)
                        else:
                            nc.vector.tensor_copy(
                                out=ob[:, k * 512:(k + 1) * 512], in_=op)
                        if k == 1:
                            nc.sync.dma_start(
                                out=out[q0:q0 + 128,
                                        (ec - 1) * 512:(ec + 1) * 512],
                                in_=ob)
                    # per-qs AV accumulation state: kt lists
                    avkts = [[kt for kt in vis
                              if not (isinstance(cls[kt], tuple)
                                      and cls[kt][0] == "T"
                                      and cls[kt][1] > qs)]
                             for qs in range(4)]
                    avseen = [0, 0, 0, 0]
                    av2 = [P.tile([128, 129], F32, name=f"av{qs}", tag=AVB[qs])
                           for qs in range(4)]
                    def emit_qs_tail(qs):
                        rl = onpool.tile([128, 1], F32, name="rl", tag="rl")
                        nc.vector.reciprocal(rl, av2[qs][:, 128:129])
                        otn = onpool.tile([128, 128], BF16, name="otn",
                                          tag="otn")
                        nc.vector.tensor_scalar_mul(otn, av2[qs][:, 0:128], rl)
                        tp2 = P.tile([128, 128], BF16, name="tp2", tag=AVB[qs])
                        nc.tensor.transpose(tp2, otn, ident)
                        if qc >= 2:  # ACT is exp-saturated in late chunks
                            nc.vector.tensor_copy(
                                out=o_cur[h][:, qs * 128:(qs + 1) * 128],
                                in_=tp2)
                        else:
                            nc.scalar.copy(
                                out=o_cur[h][:, qs * 128:(qs + 1) * 128],
                                in_=tp2)

                    def emit_av(kt, p):
                        for qs in range(4):
                            if kt not in avkts[qs]:
                                continue
                            first = avseen[qs] == 0
                            avseen[qs] += 1
                            last = avseen[qs] == len(avkts[qs])
                            nc.tensor.matmul(
                                av2[qs],
                                p[:, qs * 128:(qs + 1) * 128],
                                vTo[:, kt, :],
                                start=first, stop=last,
                                skip_group_check=True)
                            if last:
                                if qc == 0:
                                    emit_qs_tail(qs)  # frees b2 for qs3
                                else:
                                    tails.append(qs)

                    pops = {}
                    for k in range(ECH):
                        pops.setdefault((k * len(vis)) // ECH, 0)
                        pops[(k * len(vis)) // ECH] += 1
                    prev = None  # (kt, p) — AV runs one tile behind exp
                    tails = []  # qs normalize/transpose, deferred one unit
                    for i, kt in enumerate(vis):
                        if tails:
                            emit_qs_tail(tails.pop(0))
                        cl = cls[kt]
                        q0 = cl[1] * 128 if (isinstance(cl, tuple)
                                             and cl[0] == "T") else 0
                        stp = P.tile([128, 512], F32, name="st",
                                     tag="b0" if i % 2 == 0 else "b1")
                        nc.tensor.matmul(
                            stp[:, q0:512],
                            kr[:, kt * 128:(kt + 1) * 128],
                            qr[h][:, qc * 512 + q0: qc * 512 + 512],
                            start=True, stop=True, skip_group_check=True)
                        if isinstance(cl, tuple) and cl[0] == "T":
                            nc.tensor.matmul(
                                stp[:, q0:q0 + 128], triA_sb, triB_sb,
                                start=False, stop=True, skip_group_check=True)
                        elif isinstance(cl, tuple) and cl[0] == "M":
                            nc.vector.tensor_tensor(
                                out=stp, in0=stp, in1=mt_sb[:, cl[1], :],
                                op=ADD)
                        p = spool.tile([128, 512], BF16, name="p", tag="p")
                        nc.scalar.activation(
                            out=p[:, q0:512], in_=stp[:, q0:512], func=EXP)
                        if prev is not None:
                            emit_av(*prev)
                        prev = (kt, p)
                        for _ in range(pops.get(i, 0)):
                            if pend_oproj:
                                emit_op_unit(pend_oproj.pop(0))
                    emit_av(*prev)
                    while tails or pend_oproj:
                        if pend_oproj:
                            emit_op_unit(pend_oproj.pop(0))
                        if tails:
                            emit_qs_tail(tails.pop(0))
                if qc == 0:
                    cqd, sqd = pend_rope_cs
                    for dst, acc, bias in pend_ropes:
                        rope(dst, acc, cqd, sqd, bias)
            # final o-proj for the last chunk: six banks, copies alternate
            # DVE/ACT (nothing else runs here)
            FB = ["b0", "b1", "b2", "b3", "b4", "b5"]
            for qs in range(4):
                for pi, ec0 in enumerate(range(0, ECH, 2)):
                    u = qs * 4 + pi
                    oproj_pair(SC - 1, qs, ec0,
                               slot=(FB[(2 * u) % 6], FB[(2 * u + 1) % 6]),
                               acts=(False, True), split_dma=(u >= 14))

    nc.finalize()
    return nc


def _host_prep(x, mask, position_ids, wq, bq, wk, bk, wv, bv, wo, bo):
    scale = 1.0 / np.sqrt(np.float32(D))
    xT = np.ascontiguousarray(x.reshape(S, E).T).astype(BF)
    wq_s = (wq * scale).astype(np.float32)
    wk_b = wk.astype(np.float32)
    wv_b = wv.astype(np.float32)
    wo_b = wo.astype(BF)

    pos = position_ids.reshape(S).astype(np.float32)
    inv_freq = 1.0 / (10000.0 ** (np.arange(0, D, 2, dtype=np.float32) / D))
    freqs = np.outer(pos, inv_freq)                     # [S, D/2]
    emb = np.concatenate([freqs, freqs], axis=1)        # [S, D]
    cosT = np.ascontiguousarray(np.cos(emb).astype(np.float32).T)
    sin = np.sin(emb).astype(np.float32)
    sin[:, : D // 2] *= -1.0                            # sign for partition swap
    sinT = np.ascontiguousarray(sin.T)

    maskT = np.ascontiguousarray(mask.reshape(S, S).T)
    # canonical 128x128 triangle: T[k, q] = 0 if q >= k else NEG
    ktri = np.arange(128)[:, None]
    qtri = np.arange(128)[None, :]
    tri = np.where(qtri >= ktri, 0.0, NEG).astype(np.float32)
    # rank-128 factors: (triA.T @ triB)[k, q] = -s^2 (k - q) for k > q, 0 else
    sfac = np.float32(np.sqrt(1e9))
    mtri = np.arange(128)
    triA = np.where(mtri[:, None] < mtri[None, :], -sfac, 0.0).astype(BF)
    triB = np.where(mtri[:, None] >= mtri[None, :], sfac, 0.0).astype(BF)

    classes = []
    muniq = []      # unique general mask tiles
    mkeys = {}

    def mref(t):
        key = t.tobytes()
        if key not in mkeys:
            mkeys[key] = len(muniq)
            muniq.append(t)
        return ("M", mkeys[key])

    for qc in range(SC):
        row = []
        for kt in range(KT):
            t = maskT[kt * 128:(kt + 1) * 128, qc * 512:qc * 512 + 512]
            if np.all(t <= -1e8):
                row.append(SKIP)
                continue
            if np.all(t == 0.0):
                row.append(NOMASK)
                continue
            j = kt - 4 * qc
            if 0 <= j <= 3:
                q0 = j * 128
                ok = (q0 == 0 or np.all(t[:, :q0] <= -1e8))
                ok = ok and np.array_equal(t[:, q0:q0 + 128], tri)
                ok = ok and (q0 + 128 == 512 or np.all(t[:, q0 + 128:] == 0.0))
                if ok:
                    row.append(("T", j))
                    continue
            row.append(mref(t))
        if all(c == SKIP for c in row):       # fully-masked rows: keep finite
            row = [mref(maskT[kt * 128:(kt + 1) * 128,
                              qc * 512:qc * 512 + 512]) for kt in range(KT)]
        classes.append(tuple(row))
    classes = tuple(classes)

    mtiles = np.concatenate(muniq, axis=0) if muniq else None
    use_bias = bool(np.any(bq) or np.any(bk) or np.any(bv))
    return (xT, wq_s, wk_b, wv_b, wo_b, cosT, sinT, triA, triB, mtiles,
            classes, use_bias)


def kernel(x, mask, position_ids, wq, bq, wk, bk, wv, bv, wo, bo):
    (xT, wq_s, wk_b, wv_b, wo_b, cosT, sinT, triA, triB, mtiles,
     classes, use_bias) = _host_prep(
        x, mask, position_ids, wq, bq, wk, bk, wv, bv, wo, bo)

    n_mtiles = 0 if mtiles is None else mtiles.shape[0] // 128
    key = (classes, n_mtiles, use_bias)
    if key not in _build_cache:
        _build_cache[key] = _build(classes, n_mtiles, use_bias)
    nc = _build_cache[key]

    in_maps = []
    for c in range(NCORES):
        qsl = slice(c * HPC * D, (c + 1) * HPC * D)
        ksl = slice(c * D, (c + 1) * D)
        wqc = wq_s[:, qsl]
        # [q0|q1|v | q2|q3|k]: pass-A half first
        wab = np.ascontiguousarray(np.concatenate(
            [wqc[:, 0:256], wv_b[:, ksl], wqc[:, 256:512], wk_b[:, ksl]],
            axis=1)).astype(BF)
        m = {
            "xT": xT,
            "wab": wab,
            "wo": np.ascontiguousarray(wo_b[qsl, :]),
            "cos": cosT, "sinS": sinT, "triA": triA, "triB": triB,
        }
        if mtiles is not None:
            m["mtiles"] = mtiles
        if use_bias:
            m["bq"] = np.ascontiguousarray(bq[qsl]).astype(np.float32)
            m["bk"] = np.ascontiguousarray(bk[ksl]).astype(np.float32)
            m["bv"] = np.ascontiguousarray(bv[ksl]).astype(np.float32)
        in_maps.append(m)

    res = run_bass_kernel_spmd(nc, in_maps, list(range(NCORES)))
    kernel._last_results = res

    acc = res.results[0]["out"].astype(np.float32)
    for c in range(1, NCORES):
        acc = acc + res.results[c]["out"].astype(np.float32)
    acc = acc + bo[None, :]
    return acc.reshape(B, S, E).astype(np.float32)


# revision 77
# speedup vs baseline: 1.4499x; 1.0010x over previous
"""Llama GQA attention (B=1, S=2048, E=4096, H=32, KV=8, D=128) on 8 trn2 cores.

Sharding: tensor-parallel over KV groups. Core c owns kv head c and q heads
4c..4c+3: wq/wk/wv output-dim shards, wo input-dim shard. Each core computes a
partial [S, E] output (bf16); host sums the 8 partials and adds bo.

Data plane is bf16 (PE runs bf16 at 1 cycle/row for any moving width; DMA
traffic halves vs f32). PSUM accumulation stays f32; RoPE uses f32 cos/sin.
1/sqrt(D) is folded into wq on the host, so q and k share one cos/sin pair.

Per core, everything transposed [feature, seq]:
  phase 1 (per 512-seq chunk, two passes over resident x tiles so the acc
  PSUM banks double-buffer): q = wq_c.T @ x.T -> 4x [128, S]; k, v -> [128, S].
  RoPE via partition-swapped multiply (host passes sign-adjusted sin).
  v transposed via PE into vTo [k, kt, 129] with a constant 1.0 in col 128.
  phase 2: scoresT tile [k 128, q<=512] = kr.T-matmul; diagonal tiles compute
  only the visible q range and add one shared [128,128] triangle mask; Exp on
  ACT -> P bf16. AV is flipped: stationary = P[:, qs*128:+128], moving =
  vTo[:, kt, 0:129] -> av2 [q 128, 129] accumulates over kt in PSUM, col 128
  = softmax denominator for free. Per-partition reciprocal + scalar-mul
  normalize, PE-transpose back to [D, q] for the o-projection.
  phase 3: out[q, E] += o_tile.T @ wo, interleaved one chunk behind attention
  so the PE keeps busy while ACT catches up on Exp.
PSUM is managed as 8 explicit bank tags in one pool (no pool-boundary stalls):
b0-b2 proj passA / scores+..., b3-b5 proj passB / AV accumulators,
b6-b7 v-transposes / o-proj.
"""

import sys

sys.path.insert(0, "/opt/trn_rl_repo")

import numpy as np
import ml_dtypes

import concourse.bass as bass  # noqa: F401
import concourse.bacc as bacc
import concourse.mybir as mybir
import concourse.tile as tile
from concourse.bass_utils import run_bass_kernel_spmd
from concourse.masks import make_identity

F32 = mybir.dt.float32
BF16 = mybir.dt.bfloat16
ADD = mybir.AluOpType.add
MULT = mybir.AluOpType.mult
EXP = mybir.ActivationFunctionType.Exp
BF = ml_dtypes.bfloat16

B, S, E = 1, 2048, 4096
H, KV, D = 32, 8, 128
NCORES = 8
HPC = H // NCORES          # 4 q heads per core
ET = E // 128              # 32 contraction tiles
SC = S // 512              # 4 seq chunks of 512
KT = S // 128              # 16 k tiles of 128
ECH = E // 512             # 8 output E chunks
NEG = -1e9

SKIP = "S"
NOMASK = "N"

_build_cache = {}


def _build(classes, n_mtiles, use_bias):
    nc = bacc.Bacc(None, target_bir_lowering=False)

    xT = nc.declare_dram_parameter("xT", [E, S], BF16, isOutput=False)
    # wab: [q0|q1|v | q2|q3|k] so pass A's half loads first
    wab = nc.declare_dram_parameter("wab", [E, 6 * D], BF16, isOutput=False)
    wo = nc.declare_dram_parameter("wo", [HPC * D, E], BF16, isOutput=False)
    cos = nc.declare_dram_parameter("cos", [D, S], F32, isOutput=False)
    sinS = nc.declare_dram_parameter("sinS", [D, S], F32, isOutput=False)
    # causal triangle as a rank-128 product: triA.T @ triB = -1e9*(k-q)*[k>q]
    triA = nc.declare_dram_parameter("triA", [D, D], BF16, isOutput=False)
    triB = nc.declare_dram_parameter("triB", [D, D], BF16, isOutput=False)
    if n_mtiles:
        mtiles = nc.declare_dram_parameter(
            "mtiles", [n_mtiles * 128, 512], F32, isOutput=False)
    if use_bias:
        bq = nc.declare_dram_parameter("bq", [HPC * D], F32, isOutput=False)
        bk = nc.declare_dram_parameter("bk", [D], F32, isOutput=False)
        bv = nc.declare_dram_parameter("bv", [D], F32, isOutput=False)
    out = nc.declare_dram_parameter("out", [S, E], BF16, isOutput=True)

    wab_r = wab.rearrange("(t p) n -> p t n", p=128)
    wo_r = wo.rearrange("(t p) n -> p t n", p=128)
    xT_r = xT.rearrange("(t p) s -> p t s", p=128)

    with tile.TileContext(nc) as tc:
        with (
            tc.tile_pool(name="const", bufs=1) as cpool,
            tc.tile_pool(name="qkv", bufs=1) as qkvpool,
            tc.tile_pool(name="wts", bufs=1) as wpool,
            tc.tile_pool(name="xres", bufs=2) as xpool,
            tc.tile_pool(name="cs", bufs=2) as cspool,
            tc.tile_pool(name="tp", bufs=1) as tpool,
            tc.tile_pool(name="ps", bufs=6) as spool,
            tc.tile_pool(name="osb", bufs=2) as opool,
            tc.tile_pool(name="onrm", bufs=4) as onpool,
            tc.tile_pool(name="ob", bufs=4) as obpool,
            tc.tile_pool(name="psum", bufs=1, space="PSUM") as P,
        ):
            ident = cpool.tile([128, 128], BF16)
            make_identity(nc, ident)
            triA_sb = cpool.tile([128, 128], BF16)
            triB_sb = cpool.tile([128, 128], BF16)  # DMA'd after chunk-0 loads
            mt_sb = None
            if n_mtiles:
                mt_sb = cpool.tile([128, n_mtiles, 512], F32)
                nc.sync.dma_start(
                    out=mt_sb,
                    in_=mtiles.rearrange("(t p) n -> p t n", p=128))
            if use_bias:
                bq_sb = cpool.tile([128, HPC], F32)
                nc.sync.dma_start(out=bq_sb, in_=bq.rearrange("(h d) -> d h", d=128))
                bk_sb = cpool.tile([128, 1], F32)
                nc.sync.dma_start(out=bk_sb, in_=bk.rearrange("d -> d 1"))
                bv_sb = cpool.tile([128, 1], F32)
                nc.sync.dma_start(out=bv_sb, in_=bv.rearrange("d -> d 1"))

            # persistent activations
            qr = [qkvpool.tile([128, S], BF16, name=f"qr{h}", tag=f"qr{h}")
                  for h in range(HPC)]
            kr = qkvpool.tile([128, S], BF16, name="kr", tag="kr")
            vTo = qkvpool.tile([128, KT, 129], BF16, tag="vTo")  # [k%128, kt, D|1]
            nc.vector.memset(vTo[:, :, 128:129], 1.0)

            wab_sb = wpool.tile([128, ET, 6 * D], BF16)
            wo_sb = wpool.tile([128, HPC, E], BF16)

            # ---------------- phase 1: projections + RoPE + vT ----------------
            def rope(dst, acc, ct, st_, bias):
                src = acc
                if use_bias:
                    bsrc = tpool.tile([128, 512], F32, name="bsrc", tag="bsrc")
                    nc.vector.tensor_scalar_add(bsrc, acc, bias)
                    src = bsrc
                tmp = tpool.tile([128, 512], F32, name="tmp", tag="tmp")
                nc.vector.tensor_tensor(
                    out=tmp[0:64, :], in0=src[64:128, :], in1=st_[0:64, :], op=MULT)
                nc.vector.tensor_tensor(
                    out=tmp[64:128, :], in0=src[0:64, :], in1=st_[64:128, :], op=MULT)
                tmp2 = tpool.tile([128, 512], F32, name="tmp2", tag="tmp2")
                nc.vector.tensor_tensor(out=tmp2, in0=src, in1=ct, op=MULT)
                nc.vector.tensor_tensor(out=dst, in0=tmp2, in1=tmp, op=ADD)

            BA = ["b0", "b1", "b2"]
            BB = ["b3", "b4", "b5"]
            def emit_vtrans(c, vtmp):
                for j in range(4):
                    tb = "b6" if j % 2 == 0 else "b7"
                    tpsum = P.tile([128, 128], BF16, name="tp", tag=tb)
                    nc.tensor.transpose(
                        tpsum, vtmp[:, j * 128:(j + 1) * 128], ident)
                    nc.vector.tensor_copy(
                        out=vTo[:, c * 4 + j, 0:128], in_=tpsum)

            pend_vtrans = None  # chunk 0's v-transposes run in chunk 1
            for c in range(SC):
                ssl = slice(c * 512, c * 512 + 512)
                xc = xpool.tile([128, ET, 512], BF16, name="xc", tag="xc")
                # DMA in consumption order (HWDGE issue is ~625ns per
                # dma_start — few big DMAs, not many small ones)
                cq = cspool.tile([128, 512], F32, name="cq", tag="cq")
                sq = cspool.tile([128, 512], F32, name="sq", tag="sq")
                if c == 0:
                    # both weight halves per group: chunk 0 runs passes A+B
                    # interleaved so PE outpaces the DMA ramp
                    edges = [0, 1, 4, 8, 12, 16, 20, 24, 28, 32]
                    for gi in range(len(edges) - 1):
                        eg = slice(edges[gi], edges[gi + 1])
                        nc.sync.dma_start(
                            out=wab_sb[:, eg, :], in_=wab_r[:, eg, :])
                        nc.sync.dma_start(out=xc[:, eg, :], in_=xT_r[:, eg, ssl])
                        if gi == 4:
                            nc.sync.dma_start(out=cq, in_=cos[:, ssl])
                            nc.sync.dma_start(out=sq, in_=sinS[:, ssl])
                    nc.sync.dma_start(out=triA_sb, in_=triA[:, :])
                    nc.sync.dma_start(out=triB_sb, in_=triB[:, :])
                else:
                    for g in range(4):
                        eg = slice(g * 8, g * 8 + 8)
                        nc.sync.dma_start(out=xc[:, eg, :], in_=xT_r[:, eg, ssl])
                    nc.sync.dma_start(out=cq, in_=cos[:, ssl])
                    nc.sync.dma_start(out=sq, in_=sinS[:, ssl])
                    if c in (1, 2):
                        for hg in ((0, 1) if c == 1 else (2, 3)):
                            nc.sync.dma_start(
                                out=wo_sb[:, hg:hg + 1, :],
                                in_=wo_r[:, hg:hg + 1, :])

                accs = [P.tile([128, 512], F32, name=f"pa{i}", tag=BA[i])
                        for i in range(3)]
                accs2 = [P.tile([128, 512], F32, name=f"pb{i}", tag=BB[i])
                         for i in range(3)]
                def make_vtmp(acc):
                    vt = tpool.tile([128, 512], BF16, name="vtmp", tag="vtmp",
                                    bufs=2)
                    if use_bias:
                        nc.vector.tensor_scalar_add(vt, acc, bv_sb[:, 0:1])
                    else:
                        nc.scalar.copy(out=vt, in_=acc)
                    return vt

                if c == 0:
                    # single fused pass, e-major: chunk 0 is DMA-paced so the
                    # matmuls must chase the per-e loads
                    for e in range(ET):
                        st, sp = (e == 0), (e == ET - 1)
                        for ai, col in enumerate((0, 128, 256)):
                            nc.tensor.matmul(
                                accs[ai], wab_sb[:, e, col:col + 128],
                                xc[:, e, :], start=st, stop=sp)
                        for ai, col in enumerate((384, 512, 640)):
                            nc.tensor.matmul(
                                accs2[ai], wab_sb[:, e, col:col + 128],
                                xc[:, e, :], start=st, stop=sp)
                    rope(qr[0][:, ssl], accs[0], cq, sq,
                         bq_sb[:, 0:1] if use_bias else None)
                    rope(qr[1][:, ssl], accs[1], cq, sq,
                         bq_sb[:, 1:2] if use_bias else None)
                    vtmp = make_vtmp(accs[2])
                    pend_vtrans = (0, vtmp)
                    rope(kr[:, ssl], accs2[2], cq, sq,
                         bk_sb[:, 0:1] if use_bias else None)
                    rope(qr[2][:, ssl], accs2[0], cq, sq,
                         bq_sb[:, 2:3] if use_bias else None)
                    rope(qr[3][:, ssl], accs2[1], cq, sq,
                         bq_sb[:, 3:4] if use_bias else None)
                else:
                    # acc-major: each accumulator finishes early so its RoPE /
                    # copy overlaps the next accumulation instead of tailing
                    def acc_loop(acc, col):
                        for e in range(ET):
                            nc.tensor.matmul(
                                acc, wab_sb[:, e, col:col + 128], xc[:, e, :],
                                start=(e == 0), stop=(e == ET - 1))
                    acc_loop(accs[2], 256)                     # v
                    vtmp = make_vtmp(accs[2])
                    if pend_vtrans is not None:
                        emit_vtrans(*pend_vtrans)
                        pend_vtrans = None
                    acc_loop(accs[0], 0)                       # q0
                    rope(qr[0][:, ssl], accs[0], cq, sq,
                         bq_sb[:, 0:1] if use_bias else None)
                    acc_loop(accs[1], 128)                     # q1
                    rope(qr[1][:, ssl], accs[1], cq, sq,
                         bq_sb[:, 1:2] if use_bias else None)
                    emit_vtrans(c, vtmp)
                    if c == SC - 1:
                        acc_loop(accs2[2], 640)                # k first
                        rope(kr[:, ssl], accs2[2], cq, sq,
                             bk_sb[:, 0:1] if use_bias else None)
                        acc_loop(accs2[0], 384)                # q2
                        acc_loop(accs2[1], 512)                # q3
                        # defer the last two RoPEs: they would block q-chunk-0
                        # attention's DVE work (in-order engine)
                        pend_ropes = [
                            (qr[2][:, ssl], accs2[0],
                             bq_sb[:, 2:3] if use_bias else None),
                            (qr[3][:, ssl], accs2[1],
                             bq_sb[:, 3:4] if use_bias else None),
                        ]
                        pend_rope_cs = (cq, sq)
                    else:
                        acc_loop(accs2[0], 384)                # q2
                        rope(qr[2][:, ssl], accs2[0], cq, sq,
                             bq_sb[:, 2:3] if use_bias else None)
                        acc_loop(accs2[1], 512)                # q3
                        rope(qr[3][:, ssl], accs2[1], cq, sq,
                             bq_sb[:, 3:4] if use_bias else None)
                        acc_loop(accs2[2], 640)                # k
                        rope(kr[:, ssl], accs2[2], cq, sq,
                             bk_sb[:, 0:1] if use_bias else None)

            # ------------- phases 2+3: attention (+interleaved o-proj) -------
            o_bufs = [None, None]  # [qc%2] -> list of 4 o_sb tiles

            def oproj_pair(qcp, qs, ec0, slot=("b6", "b7"),
                           acts=(False, False), split_dma=False):
                """Two consecutive ec units sharing one output DMA."""
                osrc = o_bufs[qcp % 2]
                q0 = qcp * 512 + qs * 128
                ob = obpool.tile([128, 1024], BF16, name="ob", tag="ob")
                for k in range(2):
                    ec = ec0 + k
                    op = P.tile([128, 512], F32, name="op", tag=slot[k])
                    for hh in range(HPC):
                        nc.tensor.matmul(
                            op,
                            osrc[hh][:, qs * 128:(qs + 1) * 128],
                            wo_sb[:, hh, ec * 512:(ec + 1) * 512],
                            start=(hh == 0), stop=(hh == HPC - 1),
                            skip_group_check=True)
                    if acts[k]:
                        nc.scalar.copy(out=ob[:, k * 512:(k + 1) * 512], in_=op)
                    else:
                        nc.vector.tensor_copy(
                            out=ob[:, k * 512:(k + 1) * 512], in_=op)
                    if split_dma:
                        nc.sync.dma_start(
                            out=out[q0:q0 + 128, ec * 512:(ec + 1) * 512],
                            in_=ob[:, k * 512:(k + 1) * 512])
                if not split_dma:
                    nc.sync.dma_start(
                        out=out[q0:q0 + 128, ec0 * 512:(ec0 + 2) * 512], in_=ob)

            for qc in range(SC):
                # qc0 runs before the deferred chunk-3 RoPE tail frees
                # b3/b4/b5: keep it entirely off those banks (qs3 reuses b2
                # after qs0's tail releases it)
                AVB = (["b2", "b6", "b7", "b2"] if qc == 0
                       else ["b2", "b3", "b4", "b5"])
                cls = classes[qc]
                vis = [kt for kt in range(KT) if cls[kt] != SKIP]
                o_cur = [opool.tile([128, 512], BF16, name=f"o{h}", tag=f"o{h}")
                         for h in range(HPC)]
                o_bufs[qc % 2] = o_cur
                for h in range(HPC):
                    pend_oproj = list(range(ECH)) if qc > 0 else []
                    ob_half = [None]  # open ob tile for the current pair

                    def emit_op_unit(ec):
                        """One o-proj ec unit; pairs share an ob tile+DMA."""
                        osrc = o_bufs[(qc - 1) % 2]
                        q0 = (qc - 1) * 512 + h * 128
                        if ec % 2 == 0:
                            ob_half[0] = obpool.tile(
                                [128, 1024], BF16, name="ob", tag="ob")
                        ob = ob_half[0]
                        op = P.tile([128, 512], F32, name="op",
                                    tag="b6" if ec % 2 == 0 else "b7")
                        for hh in range(HPC):
                            nc.tensor.matmul(
                                op,
                                osrc[hh][:, h * 128:(h + 1) * 128],
                                wo_sb[:, hh, ec * 512:(ec + 1) * 512],
                                start=(hh == 0), stop=(hh == HPC - 1),
                                skip_group_check=True)
                        k = ec % 2
                        if qc == 1 or k == 0:
                            nc.scalar.copy(
                                out=ob[:, k * 512:(k + 1) * 512], in_=op)
                        else:
                            nc.vector.tensor_copy(
                                out=ob[:, k * 512:(k + 1) * 512], in_=op)
                        if k == 1:
                            nc.sync.dma_start(
                                out=out[q0:q0 + 128,
                                        (ec - 1) * 512:(ec + 1) * 512],
                                in_=ob)
                    # per-qs AV accumulation state: kt lists
                    avkts = [[kt for kt in vis
                              if not (isinstance(cls[kt], tuple)
                                      and cls[kt][0] == "T"
                                      and cls[kt][1] > qs)]
                             for qs in range(4)]
                    avseen = [0, 0, 0, 0]
                    av2 = [P.tile([128, 129], F32, name=f"av{qs}", tag=AVB[qs])
                           for qs in range(4)]
                    def emit_qs_tail(qs):
                        rl = onpool.tile([128, 1], F32, name="rl", tag="rl")
                        nc.vector.reciprocal(rl, av2[qs][:, 128:129])
                        otn = onpool.tile([128, 128], BF16, name="otn",
                                          tag="otn")
                        nc.vector.tensor_scalar_mul(otn, av2[qs][:, 0:128], rl)
                        tp2 = P.tile([128, 128], BF16, name="tp2", tag=AVB[qs])
                        nc.tensor.transpose(tp2, otn, ident)
                        if qc >= 2:  # ACT is exp-saturated in late chunks
                            nc.vector.tensor_copy(
                                out=o_cur[h][:, qs * 128:(qs + 1) * 128],
                                in_=tp2)
                        else:
                            nc.scalar.copy(
                                out=o_cur[h][:, qs * 128:(qs + 1) * 128],
                                in_=tp2)

                    def emit_av(kt, p):
                        for qs in range(4):
                            if kt not in avkts[qs]:
                                continue
                            first = avseen[qs] == 0
                            avseen[qs] += 1
                            last = avseen[qs] == len(avkts[qs])
                            nc.tensor.matmul(
                                av2[qs],
                                p[:, qs * 128:(qs + 1) * 128],
                                vTo[:, kt, :],
                                start=first, stop=last,
                                skip_group_check=True)
                            if last:
                                if qc == 0:
                                    emit_qs_tail(qs)  # frees b2 for qs3
                                else:
                                    tails.append(qs)

                    pops = {}
                    for k in range(ECH):
                        pops.setdefault((k * len(vis)) // ECH, 0)
                        pops[(k * len(vis)) // ECH] += 1
                    prev = None  # (kt, p) — AV runs one tile behind exp
                    tails = []  # qs normalize/transpose, deferred one unit
                    for i, kt in enumerate(vis):
                        if tails:
                            emit_qs_tail(tails.pop(0))
                        cl = cls[kt]
                        q0 = cl[1] * 128 if (isinstance(cl, tuple)
                                             and cl[0] == "T") else 0
                        stp = P.tile([128, 512], F32, name="st",
                                     tag="b0" if i % 2 == 0 else "b1")
                        nc.tensor.matmul(
                            stp[:, q0:512],
                            kr[:, kt * 128:(kt + 1) * 128],
                            qr[h][:, qc * 512 + q0: qc * 512 + 512],
                            start=True, stop=True, skip_group_check=True)
                        if isinstance(cl, tuple) and cl[0] == "T":
                            nc.tensor.matmul(
                                stp[:, q0:q0 + 128], triA_sb, triB_sb,
                                start=False, stop=True, skip_group_check=True)
                        elif isinstance(cl, tuple) and cl[0] == "M":
                            nc.vector.tensor_tensor(
                                out=stp, in0=stp, in1=mt_sb[:, cl[1], :],
                                op=ADD)
                        p = spool.tile([128, 512], BF16, name="p", tag="p")
                        nc.scalar.activation(
                            out=p[:, q0:512], in_=stp[:, q0:512], func=EXP)
                        if prev is not None:
                            emit_av(*prev)
                        prev = (kt, p)
                        for _ in range(pops.get(i, 0)):
                            if pend_oproj:
                                emit_op_unit(pend_oproj.pop(0))
                    emit_av(*prev)
                    while tails or pend_oproj:
                        if pend_oproj:
                            emit_op_unit(pend_oproj.pop(0))
                        if tails:
                            emit_qs_tail(tails.pop(0))
                if qc == 0:
                    cqd, sqd = pend_rope_cs
                    for dst, acc, bias in pend_ropes:
                        rope(dst, acc, cqd, sqd, bias)
            # final o-proj for the last chunk: six banks, copies alternate
            # DVE/ACT (nothing else runs here)
            FB = ["b0", "b1", "b2", "b3", "b4", "b5"]
            for qs in range(4):
                for pi, ec0 in enumerate(range(0, ECH, 2)):
                    u = qs * 4 + pi
                    oproj_pair(SC - 1, qs, ec0,
                               slot=(FB[(2 * u) % 6], FB[(2 * u + 1) % 6]),
                               acts=(False, True), split_dma=(u >= 14))

    nc.finalize()
    return nc


def _host_prep(x, mask, position_ids, wq, bq, wk, bk, wv, bv, wo, bo):
    scale = 1.0 / np.sqrt(np.float32(D))
    xT = np.ascontiguousarray(x.reshape(S, E).T).astype(BF)
    wq_s = (wq * scale).astype(np.float32)
    wk_b = wk.astype(np.float32)
    wv_b = wv.astype(np.float32)
    wo_b = wo.astype(BF)

    pos = position_ids.reshape(S).astype(np.float32)
    inv_freq = 1.0 / (10000.0 ** (np.arange(0, D, 2, dtype=np.float32) / D))
    freqs = np.outer(pos, inv_freq)                     # [S, D/2]
    emb = np.concatenate([freqs, freqs], axis=1)        # [S, D]
    cosT = np.ascontiguousarray(np.cos(emb).astype(np.float32).T)
    sin = np.sin(emb).astype(np.float32)
    sin[:, : D // 2] *= -1.0                            # sign for partition swap
    sinT = np.ascontiguousarray(sin.T)

    maskT = np.ascontiguousarray(mask.reshape(S, S).T)
    # canonical 128x128 triangle: T[k, q] = 0 if q >= k else NEG
    ktri = np.arange(128)[:, None]
    qtri = np.arange(128)[None, :]
    tri = np.where(qtri >= ktri, 0.0, NEG).astype(np.float32)
    # rank-128 factors: (triA.T @ triB)[k, q] = -s^2 (k - q) for k > q, 0 else
    sfac = np.float32(np.sqrt(1e9))
    mtri = np.arange(128)
    triA = np.where(mtri[:, None] < mtri[None, :], -sfac, 0.0).astype(BF)
    triB = np.where(mtri[:, None] >= mtri[None, :], sfac, 0.0).astype(BF)

    classes = []
    muniq = []      # unique general mask tiles
    mkeys = {}

    def mref(t):
        key = t.tobytes()
        if key not in mkeys:
            mkeys[key] = len(muniq)
            muniq.append(t)
        return ("M", mkeys[key])

    for qc in range(SC):
        row = []
        for kt in range(KT):
            t = maskT[kt * 128:(kt + 1) * 128, qc * 512:qc * 512 + 512]
            if np.all(t <= -1e8):
                row.append(SKIP)
                continue
            if np.all(t == 0.0):
                row.append(NOMASK)
                continue
            j = kt - 4 * qc
            if 0 <= j <= 3:
                q0 = j * 128
                ok = (q0 == 0 or np.all(t[:, :q0] <= -1e8))
                ok = ok and np.array_equal(t[:, q0:q0 + 128], tri)
                ok = ok and (q0 + 128 == 512 or np.all(t[:, q0 + 128:] == 0.0))
                if ok:
                    row.append(("T", j))
                    continue
            row.append(mref(t))
        if all(c == SKIP for c in row):       # fully-masked rows: keep finite
            row = [mref(maskT[kt * 128:(kt + 1) * 128,
                              qc * 512:qc * 512 + 512]) for kt in range(KT)]
        classes.append(tuple(row))
    classes = tuple(classes)

    mtiles = np.concatenate(muniq, axis=0) if muniq else None
    use_bias = bool(np.any(bq) or np.any(bk) or np.any(bv))
    return (xT, wq_s, wk_b, wv_b, wo_b, cosT, sinT, triA, triB, mtiles,
            classes, use_bias)


def kernel(x, mask, position_ids, wq, bq, wk, bk, wv, bv, wo, bo):
    (xT, wq_s, wk_b, wv_b, wo_b, cosT, sinT, triA, triB, mtiles,
     classes, use_bias) = _host_prep(
        x, mask, position_ids, wq, bq, wk, bk, wv, bv, wo, bo)

    n_mtiles = 0 if mtiles is None else mtiles.shape[0] // 128
    key = (classes, n_mtiles, use_bias)
    if key not in _build_cache:
        _build_cache[key] = _build(classes, n_mtiles, use_bias)
    nc = _build_cache[key]

    in_maps = []
    for c in range(NCORES):
        qsl = slice(c * HPC * D, (c + 1) * HPC * D)
        ksl = slice(c * D, (c + 1) * D)
        wqc = wq_s[:, qsl]
        # [q0|q1|v | q2|q3|k]: pass-A half first
        wab = np.ascontiguousarray(np.concatenate(
            [wqc[:, 0:256], wv_b[:, ksl], wqc[:, 256:512], wk_b[:, ksl]],
            axis=1)).astype(BF)
        m = {
            "xT": xT,
            "wab": wab,
            "wo": np.ascontiguousarray(wo_b[qsl, :]),
            "cos": cosT, "sinS": sinT, "triA": triA, "triB": triB,
        }
        if mtiles is not None:
            m["mtiles"] = mtiles
        if use_bias:
            m["bq"] = np.ascontiguousarray(bq[qsl]).astype(np.float32)
            m["bk"] = np.ascontiguousarray(bk[ksl]).astype(np.float32)
            m["bv"] = np.ascontiguousarray(bv[ksl]).astype(np.float32)
        in_maps.append(m)

    res = run_bass_kernel_spmd(nc, in_maps, list(range(NCORES)))
    kernel._last_results = res

    acc = res.results[0]["out"].astype(np.float32)
    for c in range(1, NCORES):
        acc = acc + res.results[c]["out"].astype(np.float32)
    acc = acc + bo[None, :]
    return acc.reshape(B, S, E).astype(np.float32)


# revision 80
# speedup vs baseline: 1.4512x; 1.0009x over previous
"""Llama GQA attention (B=1, S=2048, E=4096, H=32, KV=8, D=128) on 8 trn2 cores.

Sharding: tensor-parallel over KV groups. Core c owns kv head c and q heads
4c..4c+3: wq/wk/wv output-dim shards, wo input-dim shard. Each core computes a
partial [S, E] output (bf16); host sums the 8 partials and adds bo.

Data plane is bf16 (PE runs bf16 at 1 cycle/row for any moving width; DMA
traffic halves vs f32). PSUM accumulation stays f32; RoPE uses f32 cos/sin.
1/sqrt(D) is folded into wq on the host, so q and k share one cos/sin pair.

Per core, everything transposed [feature, seq]:
  phase 1 (per 512-seq chunk, two passes over resident x tiles so the acc
  PSUM banks double-buffer): q = wq_c.T @ x.T -> 4x [128, S]; k, v -> [128, S].
  RoPE via partition-swapped multiply (host passes sign-adjusted sin).
  v transposed via PE into vTo [k, kt, 129] with a constant 1.0 in col 128.
  phase 2: scoresT tile [k 128, q<=512] = kr.T-matmul; diagonal tiles compute
  only the visible q range and add one shared [128,128] triangle mask; Exp on
  ACT -> P bf16. AV is flipped: stationary = P[:, qs*128:+128], moving =
  vTo[:, kt, 0:129] -> av2 [q 128, 129] accumulates over kt in PSUM, col 128
  = softmax denominator for free. Per-partition reciprocal + scalar-mul
  normalize, PE-transpose back to [D, q] for the o-projection.
  phase 3: out[q, E] += o_tile.T @ wo, interleaved one chunk behind attention
  so the PE keeps busy while ACT catches up on Exp.
PSUM is managed as 8 explicit bank tags in one pool (no pool-boundary stalls):
b0-b2 proj passA / scores+..., b3-b5 proj passB / AV accumulators,
b6-b7 v-transposes / o-proj.
"""

import sys

sys.path.insert(0, "/opt/trn_rl_repo")

import numpy as np
import ml_dtypes

import concourse.bass as bass  # noqa: F401
import concourse.bacc as bacc
import concourse.mybir as mybir
import concourse.tile as tile
from concourse.bass_utils import run_bass_kernel_spmd
from concourse.masks import make_identity

F32 = mybir.dt.float32
BF16 = mybir.dt.bfloat16
ADD = mybir.AluOpType.add
MULT = mybir.AluOpType.mult
EXP = mybir.ActivationFunctionType.Exp
BF = ml_dtypes.bfloat16

B, S, E = 1, 2048, 4096
H, KV, D = 32, 8, 128
NCORES = 8
HPC = H // NCORES          # 4 q heads per core
ET = E // 128              # 32 contraction tiles
SC = S // 512              # 4 seq chunks of 512
KT = S // 128              # 16 k tiles of 128
ECH = E // 512             # 8 output E chunks
NEG = -1e9

SKIP = "S"
NOMASK = "N"

_build_cache = {}


def _build(classes, n_mtiles, use_bias):
    nc = bacc.Bacc(None, target_bir_lowering=False)

    xT = nc.declare_dram_parameter("xT", [E, S], BF16, isOutput=False)
    # wab: [q0|q1|v | q2|q3|k] so pass A's half loads first
    wab = nc.declare_dram_parameter("wab", [E, 6 * D], BF16, isOutput=False)
    wo = nc.declare_dram_parameter("wo", [HPC * D, E], BF16, isOutput=False)
    cos = nc.declare_dram_parameter("cos", [D, S], F32, isOutput=False)
    sinS = nc.declare_dram_parameter("sinS", [D, S], F32, isOutput=False)
    # causal triangle as a rank-128 product: triA.T @ triB = -1e9*(k-q)*[k>q]
    triA = nc.declare_dram_parameter("triA", [D, D], BF16, isOutput=False)
    triB = nc.declare_dram_parameter("triB", [D, D], BF16, isOutput=False)
    if n_mtiles:
        mtiles = nc.declare_dram_parameter(
            "mtiles", [n_mtiles * 128, 512], F32, isOutput=False)
    if use_bias:
        bq = nc.declare_dram_parameter("bq", [HPC * D], F32, isOutput=False)
        bk = nc.declare_dram_parameter("bk", [D], F32, isOutput=False)
        bv = nc.declare_dram_parameter("bv", [D], F32, isOutput=False)
    out = nc.declare_dram_parameter("out", [S, E], BF16, isOutput=True)

    wab_r = wab.rearrange("(t p) n -> p t n", p=128)
    wo_r = wo.rearrange("(t p) n -> p t n", p=128)
    xT_r = xT.rearrange("(t p) s -> p t s", p=128)

    with tile.TileContext(nc) as tc:
        with (
            tc.tile_pool(name="const", bufs=1) as cpool,
            tc.tile_pool(name="qkv", bufs=1) as qkvpool,
            tc.tile_pool(name="wts", bufs=1) as wpool,
            tc.tile_pool(name="xres", bufs=2) as xpool,
            tc.tile_pool(name="cs", bufs=2) as cspool,
            tc.tile_pool(name="tp", bufs=1) as tpool,
            tc.tile_pool(name="ps", bufs=6) as spool,
            tc.tile_pool(name="osb", bufs=2) as opool,
            tc.tile_pool(name="onrm", bufs=4) as onpool,
            tc.tile_pool(name="ob", bufs=4) as obpool,
            tc.tile_pool(name="psum", bufs=1, space="PSUM") as P,
        ):
            ident = cpool.tile([128, 128], BF16)
            make_identity(nc, ident)
            triA_sb = cpool.tile([128, 128], BF16)
            triB_sb = cpool.tile([128, 128], BF16)  # DMA'd after chunk-0 loads
            mt_sb = None
            if n_mtiles:
                mt_sb = cpool.tile([128, n_mtiles, 512], F32)
                nc.sync.dma_start(
                    out=mt_sb,
                    in_=mtiles.rearrange("(t p) n -> p t n", p=128))
            if use_bias:
                bq_sb = cpool.tile([128, HPC], F32)
                nc.sync.dma_start(out=bq_sb, in_=bq.rearrange("(h d) -> d h", d=128))
                bk_sb = cpool.tile([128, 1], F32)
                nc.sync.dma_start(out=bk_sb, in_=bk.rearrange("d -> d 1"))
                bv_sb = cpool.tile([128, 1], F32)
                nc.sync.dma_start(out=bv_sb, in_=bv.rearrange("d -> d 1"))

            # persistent activations
            qr = [qkvpool.tile([128, S], BF16, name=f"qr{h}", tag=f"qr{h}")
                  for h in range(HPC)]
            kr = qkvpool.tile([128, S], BF16, name="kr", tag="kr")
            vTo = qkvpool.tile([128, KT, 129], BF16, tag="vTo")  # [k%128, kt, D|1]
            nc.vector.memset(vTo[:, :, 128:129], 1.0)

            wab_sb = wpool.tile([128, ET, 6 * D], BF16)
            wo_sb = wpool.tile([128, HPC, E], BF16)

            # ---------------- phase 1: projections + RoPE + vT ----------------
            def rope(dst, acc, ct, st_, bias):
                src = acc
                if use_bias:
                    bsrc = tpool.tile([128, 512], F32, name="bsrc", tag="bsrc")
                    nc.vector.tensor_scalar_add(bsrc, acc, bias)
                    src = bsrc
                tmp = tpool.tile([128, 512], F32, name="tmp", tag="tmp")
                nc.vector.tensor_tensor(
                    out=tmp[0:64, :], in0=src[64:128, :], in1=st_[0:64, :], op=MULT)
                nc.vector.tensor_tensor(
                    out=tmp[64:128, :], in0=src[0:64, :], in1=st_[64:128, :], op=MULT)
                tmp2 = tpool.tile([128, 512], F32, name="tmp2", tag="tmp2")
                nc.vector.tensor_tensor(out=tmp2, in0=src, in1=ct, op=MULT)
                nc.vector.tensor_tensor(out=dst, in0=tmp2, in1=tmp, op=ADD)

            BA = ["b0", "b1", "b2"]
            BB = ["b3", "b4", "b5"]
            def emit_vtrans(c, vtmp):
                for j in range(4):
                    tb = "b6" if j % 2 == 0 else "b7"
                    tpsum = P.tile([128, 128], BF16, name="tp", tag=tb)
                    nc.tensor.transpose(
                        tpsum, vtmp[:, j * 128:(j + 1) * 128], ident)
                    nc.vector.tensor_copy(
                        out=vTo[:, c * 4 + j, 0:128], in_=tpsum)

            pend_vtrans = None  # chunk 0's v-transposes run in chunk 1
            for c in range(SC):
                ssl = slice(c * 512, c * 512 + 512)
                xc = xpool.tile([128, ET, 512], BF16, name="xc", tag="xc")
                # DMA in consumption order (HWDGE issue is ~625ns per
                # dma_start — few big DMAs, not many small ones)
                cq = cspool.tile([128, 512], F32, name="cq", tag="cq")
                sq = cspool.tile([128, 512], F32, name="sq", tag="sq")
                if c == 0:
                    # both weight halves per group: chunk 0 runs passes A+B
                    # interleaved so PE outpaces the DMA ramp
                    edges = [0, 1, 4, 8, 12, 16, 20, 24, 28, 32]
                    for gi in range(len(edges) - 1):
                        eg = slice(edges[gi], edges[gi + 1])
                        nc.sync.dma_start(
                            out=wab_sb[:, eg, :], in_=wab_r[:, eg, :])
                        nc.sync.dma_start(out=xc[:, eg, :], in_=xT_r[:, eg, ssl])
                        if gi == 4:
                            nc.sync.dma_start(out=cq, in_=cos[:, ssl])
                            nc.sync.dma_start(out=sq, in_=sinS[:, ssl])
                    nc.sync.dma_start(out=triA_sb, in_=triA[:, :])
                    nc.sync.dma_start(out=triB_sb, in_=triB[:, :])
                else:
                    for g in range(4):
                        eg = slice(g * 8, g * 8 + 8)
                        nc.sync.dma_start(out=xc[:, eg, :], in_=xT_r[:, eg, ssl])
                    nc.sync.dma_start(out=cq, in_=cos[:, ssl])
                    nc.sync.dma_start(out=sq, in_=sinS[:, ssl])
                    if c in (1, 2):
                        for hg in ((0, 1) if c == 1 else (2, 3)):
                            nc.sync.dma_start(
                                out=wo_sb[:, hg:hg + 1, :],
                                in_=wo_r[:, hg:hg + 1, :])

                accs = [P.tile([128, 512], F32, name=f"pa{i}", tag=BA[i])
                        for i in range(3)]
                accs2 = [P.tile([128, 512], F32, name=f"pb{i}", tag=BB[i])
                         for i in range(3)]
                def make_vtmp(acc):
                    vt = tpool.tile([128, 512], BF16, name="vtmp", tag="vtmp",
                                    bufs=2)
                    if use_bias:
                        nc.vector.tensor_scalar_add(vt, acc, bv_sb[:, 0:1])
                    else:
                        nc.scalar.copy(out=vt, in_=acc)
                    return vt

                if c == 0:
                    # single fused pass, e-major: chunk 0 is DMA-paced so the
                    # matmuls must chase the per-e loads
                    for e in range(ET):
                        st, sp = (e == 0), (e == ET - 1)
                        for ai, col in enumerate((0, 128, 256)):
                            nc.tensor.matmul(
                                accs[ai], wab_sb[:, e, col:col + 128],
                                xc[:, e, :], start=st, stop=sp)
                        for ai, col in enumerate((384, 512, 640)):
                            nc.tensor.matmul(
                                accs2[ai], wab_sb[:, e, col:col + 128],
                                xc[:, e, :], start=st, stop=sp)
                    rope(qr[0][:, ssl], accs[0], cq, sq,
                         bq_sb[:, 0:1] if use_bias else None)
                    rope(qr[1][:, ssl], accs[1], cq, sq,
                         bq_sb[:, 1:2] if use_bias else None)
                    vtmp = make_vtmp(accs[2])
                    pend_vtrans = (0, vtmp)
                    rope(kr[:, ssl], accs2[2], cq, sq,
                         bk_sb[:, 0:1] if use_bias else None)
                    rope(qr[2][:, ssl], accs2[0], cq, sq,
                         bq_sb[:, 2:3] if use_bias else None)
                    rope(qr[3][:, ssl], accs2[1], cq, sq,
                         bq_sb[:, 3:4] if use_bias else None)
                else:
                    # acc-major: each accumulator finishes early so its RoPE /
                    # copy overlaps the next accumulation instead of tailing
                    def acc_loop(acc, col):
                        for e in range(ET):
                            nc.tensor.matmul(
                                acc, wab_sb[:, e, col:col + 128], xc[:, e, :],
                                start=(e == 0), stop=(e == ET - 1))
                    acc_loop(accs[2], 256)                     # v
                    vtmp = make_vtmp(accs[2])
                    if pend_vtrans is not None:
                        emit_vtrans(*pend_vtrans)
                        pend_vtrans = None
                    acc_loop(accs[0], 0)                       # q0
                    rope(qr[0][:, ssl], accs[0], cq, sq,
                         bq_sb[:, 0:1] if use_bias else None)
                    acc_loop(accs[1], 128)                     # q1
                    rope(qr[1][:, ssl], accs[1], cq, sq,
                         bq_sb[:, 1:2] if use_bias else None)
                    emit_vtrans(c, vtmp)
                    if c == SC - 1:
                        acc_loop(accs2[2], 640)                # k first
                        rope(kr[:, ssl], accs2[2], cq, sq,
                             bk_sb[:, 0:1] if use_bias else None)
                        acc_loop(accs2[0], 384)                # q2
                        acc_loop(accs2[1], 512)                # q3
                        # defer the last two RoPEs: they would block q-chunk-0
                        # attention's DVE work (in-order engine)
                        pend_ropes = [
                            (qr[2][:, ssl], accs2[0],
                             bq_sb[:, 2:3] if use_bias else None),
                            (qr[3][:, ssl], accs2[1],
                             bq_sb[:, 3:4] if use_bias else None),
                        ]
                        pend_rope_cs = (cq, sq)
                    else:
                        acc_loop(accs2[0], 384)                # q2
                        rope(qr[2][:, ssl], accs2[0], cq, sq,
                             bq_sb[:, 2:3] if use_bias else None)
                        acc_loop(accs2[1], 512)                # q3
                        rope(qr[3][:, ssl], accs2[1], cq, sq,
                             bq_sb[:, 3:4] if use_bias else None)
                        acc_loop(accs2[2], 640)                # k
                        rope(kr[:, ssl], accs2[2], cq, sq,
                             bk_sb[:, 0:1] if use_bias else None)

            # ------------- phases 2+3: attention (+interleaved o-proj) -------
            o_bufs = [None, None]  # [qc%2] -> list of 4 o_sb tiles

            def oproj_pair(qcp, qs, ec0, slot=("b6", "b7"),
                           acts=(False, False), split_dma=False):
                """Two consecutive ec units sharing one output DMA."""
                osrc = o_bufs[qcp % 2]
                q0 = qcp * 512 + qs * 128
                ob = obpool.tile([128, 1024], BF16, name="ob", tag="ob")
                for k in range(2):
                    ec = ec0 + k
                    op = P.tile([128, 512], F32, name="op", tag=slot[k])
                    for hh in range(HPC):
                        nc.tensor.matmul(
                            op,
                            osrc[hh][:, qs * 128:(qs + 1) * 128],
                            wo_sb[:, hh, ec * 512:(ec + 1) * 512],
                            start=(hh == 0), stop=(hh == HPC - 1),
                            skip_group_check=True)
                    if acts[k]:
                        nc.scalar.copy(out=ob[:, k * 512:(k + 1) * 512], in_=op)
                    else:
                        nc.vector.tensor_copy(
                            out=ob[:, k * 512:(k + 1) * 512], in_=op)
                    if split_dma:
                        nc.sync.dma_start(
                            out=out[q0:q0 + 128, ec * 512:(ec + 1) * 512],
                            in_=ob[:, k * 512:(k + 1) * 512])
                if not split_dma:
                    nc.sync.dma_start(
                        out=out[q0:q0 + 128, ec0 * 512:(ec0 + 2) * 512], in_=ob)

            for qc in range(SC):
                # qc0 runs before the deferred chunk-3 RoPE tail frees
                # b3/b4/b5: keep it entirely off those banks (qs3 reuses b2
                # after qs0's tail releases it)
                AVB = (["b2", "b6", "b7", "b2"] if qc == 0
                       else ["b2", "b3", "b4", "b5"])
                cls = classes[qc]
                vis = [kt for kt in range(KT) if cls[kt] != SKIP]
                o_cur = [opool.tile([128, 512], BF16, name=f"o{h}", tag=f"o{h}")
                         for h in range(HPC)]
                o_bufs[qc % 2] = o_cur
                for h in range(HPC):
                    pend_oproj = list(range(ECH)) if qc > 0 else []
                    ob_half = [None]  # open ob tile for the current pair

                    def emit_op_unit(ec):
                        """One o-proj ec unit; pairs share an ob tile+DMA."""
                        osrc = o_bufs[(qc - 1) % 2]
                        q0 = (qc - 1) * 512 + h * 128
                        if ec % 2 == 0:
                            ob_half[0] = obpool.tile(
                                [128, 1024], BF16, name="ob", tag="ob")
                        ob = ob_half[0]
                        op = P.tile([128, 512], F32, name="op",
                                    tag="b6" if ec % 2 == 0 else "b7")
                        for hh in range(HPC):
                            nc.tensor.matmul(
                                op,
                                osrc[hh][:, h * 128:(h + 1) * 128],
                                wo_sb[:, hh, ec * 512:(ec + 1) * 512],
                                start=(hh == 0), stop=(hh == HPC - 1),
                                skip_group_check=True)
                        k = ec % 2
                        if k == 0:
                            nc.scalar.copy(
                                out=ob[:, k * 512:(k + 1) * 512], in_=op)
                        else:
                            nc.vector.tensor_copy(
                                out=ob[:, k * 512:(k + 1) * 512], in_=op)
                        if k == 1:
                            nc.sync.dma_start(
                                out=out[q0:q0 + 128,
                                        (ec - 1) * 512:(ec + 1) * 512],
                                in_=ob)
                    # per-qs AV accumulation state: kt lists
                    avkts = [[kt for kt in vis
                              if not (isinstance(cls[kt], tuple)
                                      and cls[kt][0] == "T"
                                      and cls[kt][1] > qs)]
                             for qs in range(4)]
                    avseen = [0, 0, 0, 0]
                    av2 = [P.tile([128, 129], F32, name=f"av{qs}", tag=AVB[qs])
                           for qs in range(4)]
                    def emit_qs_tail(qs):
                        rl = onpool.tile([128, 1], F32, name="rl", tag="rl")
                        nc.vector.reciprocal(rl, av2[qs][:, 128:129])
                        otn = onpool.tile([128, 128], BF16, name="otn",
                                          tag="otn")
                        nc.vector.tensor_scalar_mul(otn, av2[qs][:, 0:128], rl)
                        tp2 = P.tile([128, 128], BF16, name="tp2", tag=AVB[qs])
                        nc.tensor.transpose(tp2, otn, ident)
                        if qc >= 2:  # ACT is exp-saturated in late chunks
                            nc.vector.tensor_copy(
                                out=o_cur[h][:, qs * 128:(qs + 1) * 128],
                                in_=tp2)
                        else:
                            nc.scalar.copy(
                                out=o_cur[h][:, qs * 128:(qs + 1) * 128],
                                in_=tp2)

                    def emit_av(kt, p):
                        for qs in range(4):
                            if kt not in avkts[qs]:
                                continue
                            first = avseen[qs] == 0
                            avseen[qs] += 1
                            last = avseen[qs] == len(avkts[qs])
                            nc.tensor.matmul(
                                av2[qs],
                                p[:, qs * 128:(qs + 1) * 128],
                                vTo[:, kt, :],
                                start=first, stop=last,
                                skip_group_check=True)
                            if last:
                                if qc == 0:
                                    emit_qs_tail(qs)  # frees b2 for qs3
                                else:
                                    tails.append(qs)

                    pops = {}
                    for k in range(ECH):
                        pops.setdefault((k * len(vis)) // ECH, 0)
                        pops[(k * len(vis)) // ECH] += 1
                    prev = None  # (kt, p) — AV runs one tile behind exp
                    tails = []  # qs normalize/transpose, deferred one unit
                    for i, kt in enumerate(vis):
                        if tails:
                            emit_qs_tail(tails.pop(0))
                        cl = cls[kt]
                        q0 = cl[1] * 128 if (isinstance(cl, tuple)
                                             and cl[0] == "T") else 0
                        stp = P.tile([128, 512], F32, name="st",
                                     tag="b0" if i % 2 == 0 else "b1")
                        nc.tensor.matmul(
                            stp[:, q0:512],
                            kr[:, kt * 128:(kt + 1) * 128],
                            qr[h][:, qc * 512 + q0: qc * 512 + 512],
                            start=True, stop=True, skip_group_check=True)
                        if isinstance(cl, tuple) and cl[0] == "T":
                            nc.tensor.matmul(
                                stp[:, q0:q0 + 128], triA_sb, triB_sb,
                                start=False, stop=True, skip_group_check=True)
                        elif isinstance(cl, tuple) and cl[0] == "M":
                            nc.vector.tensor_tensor(
                                out=stp, in0=stp, in1=mt_sb[:, cl[1], :],
                                op=ADD)
                        p = spool.tile([128, 512], BF16, name="p", tag="p")
                        nc.scalar.activation(
                            out=p[:, q0:512], in_=stp[:, q0:512], func=EXP)
                        if prev is not None:
                            emit_av(*prev)
                        prev = (kt, p)
                        for _ in range(pops.get(i, 0)):
                            if pend_oproj:
                                emit_op_unit(pend_oproj.pop(0))
                    emit_av(*prev)
                    while tails or pend_oproj:
                        if pend_oproj:
                            emit_op_unit(pend_oproj.pop(0))
                        if tails:
                            emit_qs_tail(tails.pop(0))
                if qc == 0:
                    cqd, sqd = pend_rope_cs
                    for dst, acc, bias in pend_ropes:
                        rope(dst, acc, cqd, sqd, bias)
            # final o-proj for the last chunk: six banks, copies alternate
            # DVE/ACT (nothing else runs here)
            FB = ["b0", "b1", "b2", "b3", "b4", "b5"]
            for qs in range(4):
                for pi, ec0 in enumerate(range(0, ECH, 2)):
                    u = qs * 4 + pi
                    oproj_pair(SC - 1, qs, ec0,
                               slot=(FB[(2 * u) % 6], FB[(2 * u + 1) % 6]),
                               acts=(False, True), split_dma=(u >= 14))

    nc.finalize()
    return nc


def _host_prep(x, mask, position_ids, wq, bq, wk, bk, wv, bv, wo, bo):
    scale = 1.0 / np.sqrt(np.float32(D))
    xT = np.ascontiguousarray(x.reshape(S, E).T).astype(BF)
    wq_s = (wq * scale).astype(np.float32)
    wk_b = wk.astype(np.float32)
    wv_b = wv.astype(np.float32)
    wo_b = wo.astype(BF)

    pos = position_ids.reshape(S).astype(np.float32)
    inv_freq = 1.0 / (10000.0 ** (np.arange(0, D, 2, dtype=np.float32) / D))
    freqs = np.outer(pos, inv_freq)                     # [S, D/2]
    emb = np.concatenate([freqs, freqs], axis=1)        # [S, D]
    cosT = np.ascontiguousarray(np.cos(emb).astype(np.float32).T)
    sin = np.sin(emb).astype(np.float32)
    sin[:, : D // 2] *= -1.0                            # sign for partition swap
    sinT = np.ascontiguousarray(sin.T)

    maskT = np.ascontiguousarray(mask.reshape(S, S).T)
    # canonical 128x128 triangle: T[k, q] = 0 if q >= k else NEG
    ktri = np.arange(128)[:, None]
    qtri = np.arange(128)[None, :]
    tri = np.where(qtri >= ktri, 0.0, NEG).astype(np.float32)
    # rank-128 factors: (triA.T @ triB)[k, q] = -s^2 (k - q) for k > q, 0 else
    sfac = np.float32(np.sqrt(1e9))
    mtri = np.arange(128)
    triA = np.where(mtri[:, None] < mtri[None, :], -sfac, 0.0).astype(BF)
    triB = np.where(mtri[:, None] >= mtri[None, :], sfac, 0.0).astype(BF)

    classes = []
    muniq = []      # unique general mask tiles
    mkeys = {}

    def mref(t):
        key = t.tobytes()
        if key not in mkeys:
            mkeys[key] = len(muniq)
            muniq.append(t)
        return ("M", mkeys[key])

    for qc in range(SC):
        row = []
        for kt in range(KT):
            t = maskT[kt * 128:(kt + 1) * 128, qc * 512:qc * 512 + 512]
            if np.all(t <= -1e8):
                row.append(SKIP)
                continue
            if np.all(t == 0.0):
                row.append(NOMASK)
                continue
            j = kt - 4 * qc
            if 0 <= j <= 3:
                q0 = j * 128
                ok = (q0 == 0 or np.all(t[:, :q0] <= -1e8))
                ok = ok and np.array_equal(t[:, q0:q0 + 128], tri)
                ok = ok and (q0 + 128 == 512 or np.all(t[:, q0 + 128:] == 0.0))
                if ok:
                    row.append(("T", j))
                    continue
            row.append(mref(t))
        if all(c == SKIP for c in row):       # fully-masked rows: keep finite
            row = [mref(maskT[kt * 128:(kt + 1) * 128,
                              qc * 512:qc * 512 + 512]) for kt in range(KT)]
        classes.append(tuple(row))
    classes = tuple(classes)

    mtiles = np.concatenate(muniq, axis=0) if muniq else None
    use_bias = bool(np.any(bq) or np.any(bk) or np.any(bv))
    return (xT, wq_s, wk_b, wv_b, wo_b, cosT, sinT, triA, triB, mtiles,
            classes, use_bias)


def kernel(x, mask, position_ids, wq, bq, wk, bk, wv, bv, wo, bo):
    (xT, wq_s, wk_b, wv_b, wo_b, cosT, sinT, triA, triB, mtiles,
     classes, use_bias) = _host_prep(
        x, mask, position_ids, wq, bq, wk, bk, wv, bv, wo, bo)

    n_mtiles = 0 if mtiles is None else mtiles.shape[0] // 128
    key = (classes, n_mtiles, use_bias)
    if key not in _build_cache:
        _build_cache[key] = _build(classes, n_mtiles, use_bias)
    nc = _build_cache[key]

    in_maps = []
    for c in range(NCORES):
        qsl = slice(c * HPC * D, (c + 1) * HPC * D)
        ksl = slice(c * D, (c + 1) * D)
        wqc = wq_s[:, qsl]
        # [q0|q1|v | q2|q3|k]: pass-A half first
        wab = np.ascontiguousarray(np.concatenate(
            [wqc[:, 0:256], wv_b[:, ksl], wqc[:, 256:512], wk_b[:, ksl]],
            axis=1)).astype(BF)
        m = {
            "xT": xT,
            "wab": wab,
            "wo": np.ascontiguousarray(wo_b[qsl, :]),
            "cos": cosT, "sinS": sinT, "triA": triA, "triB": triB,
        }
        if mtiles is not None:
            m["mtiles"] = mtiles
        if use_bias:
            m["bq"] = np.ascontiguousarray(bq[qsl]).astype(np.float32)
            m["bk"] = np.ascontiguousarray(bk[ksl]).astype(np.float32)
            m["bv"] = np.ascontiguousarray(bv[ksl]).astype(np.float32)
        in_maps.append(m)

    res = run_bass_kernel_spmd(nc, in_maps, list(range(NCORES)))
    kernel._last_results = res

    acc = res.results[0]["out"].astype(np.float32)
    for c in range(1, NCORES):
        acc = acc + res.results[c]["out"].astype(np.float32)
    acc = acc + bo[None, :]
    return acc.reshape(B, S, E).astype(np.float32)
